# revision 18
# baseline (speedup 1.0000x reference)
"""Trainium2 Bass kernel for nn_Encoder (FSPool set encoder).

Computation per event b (8192 events, data-parallel over 8 cores):
  h = relu(x[b].reshape(128,4) @ W1 + b1)        # per-particle MLP
  h = relu(h @ W2 + b2)
  z = h @ W3 (+ b3)                              # [128 particles, 32 ch]
  z_sorted = sort_desc(z.T, axis=-1)             # per-channel sort over particles
  pooled[c] = sum_p z_sorted[c,p] * w[c,p]       # rank-weighted pool
  mus = pooled[::2]; logvars = pooled[1::2]
  samples = mus + eps * exp(0.5*logvars)

Device layout (per core, 1024 events, variable-size supertiles):
  - MLP on TensorE (hidden on partitions); relu+cast on ACT in 2-group
    (1024-col) PSUM tiles; z materialized channel-major via matmul3 with
    tile_position: partition = 32*(e%4)+c.
  - Per-row descending 128-sort: Batcher odd-even merge-sort (1471
    comparators vs bitonic's 1792) on the DVE in fp16, event "slots"
    packed along the inner free axis for 2x DVE mode. Inner merge passes
    touch only [d, m-d) of each block; untouched rows ping-pong via 4x
    tensor_copy.
  - Rank-weighted pooling (z_sorted*w then 7-level binary-tree sum over
    ranks) runs on GPSIMD, which has no other mid-kernel work; the last
    supertile's pooling runs on the DVE so the kernel doesn't drain
    waiting on the slower engine.
  - b3 never enters the sort: its pooled contribution is a per-partition
    offset added to the stage at the end.
"""

import os
import numpy as np

NCORES = 8
B = 8192
P = 128          # particles per event (set size)
F = 4            # input features per particle
H = 128          # hidden width
C = 32           # 2*LATENT pooled channels
LAT = 16
NPIECES = 20

E = B // NCORES          # events per core
ST_E = 128               # max events per supertile
NG = ST_E // 4           # max groups (slots) per supertile: 32
GALL = E // 4            # total groups per core (stage columns): 256

_BUILT = None
LAST_RESULTS = None      # test harness can inspect exec_time_ns / profile


def _chunks(e_total):
    """Event counts per super-tile: small head/tail tiles shorten the
    pipeline ramp-in and drain."""
    if e_total >= 8 * ST_E:
        q3 = 3 * ST_E // 4
        mid = (e_total - ST_E // 2 - 2 * q3) // ST_E
        return [ST_E // 2, q3] + [ST_E] * mid + [q3]
    out = []
    left = e_total
    while left > 0:
        c = min(ST_E, left)
        out.append(c)
        left -= c
    return out


def _fspool_interp_matrix():
    """M [21, 128] with w_table = pool_weight @ M (matches reference math)."""
    pos = (np.arange(P, dtype=np.float32) / np.float32(P - 1)) * np.float32(NPIECES)
    idx = np.clip(pos.astype(np.int32), 0, NPIECES)
    frac = pos - idx.astype(np.float32)
    M = np.zeros((NPIECES + 1, P), dtype=np.float32)
    for p in range(P):
        i = int(idx[p])
        M[i, p] += np.float32(1.0) - frac[p]
        M[min(i + 1, NPIECES), p] += frac[p]
    return M


def _batcher_passes(n=P, skip_final_d1=True):
    """(kind, m, d) pass list for Batcher odd-even mergesort of n.

    With skip_final_d1, the last cleanup pass of the final merge is
    dropped: the result is sorted except that pairs (2k+1, 2k+2) may be
    swapped. The pooling weight table is pair-averaged over exactly
    those pairs (host side), which makes the pooled sum invariant to
    the missing pass up to the (tiny) within-pair weight variation."""
    out = []
    m = 2
    while m <= n:
        out.append(("first", m, m // 2))
        d = m // 4
        while d >= 1:
            if not (skip_final_d1 and m == n and d == 1):
                out.append(("inner", m, d))
            d //= 2
        m *= 2
    return out


def _emit_batcher(v, OP, zA, zB, n=P):
    """Descending Batcher sort of the position axis of zA [128, P, s].
    Ping-pongs zA/zB every pass (28 passes, even -> result in zA)."""
    cur, other = zA, zB
    for kind, m, d in _batcher_passes(n):
        va = cur[:].rearrange("p (nb m) s -> p nb m s", m=m)
        vb = other[:].rearrange("p (nb m) s -> p nb m s", m=m)
        if kind == "first":
            v.tensor_tensor(out=vb[:, :, 0:d, :], in0=va[:, :, 0:d, :],
                            in1=va[:, :, d:m, :], op=OP.max)
            v.tensor_tensor(out=vb[:, :, d:m, :], in0=va[:, :, 0:d, :],
                            in1=va[:, :, d:m, :], op=OP.min)
        else:
            nruns = m // (2 * d) - 1
            # runs start at d + 2dt: view run axis explicitly
            ra = va[:].rearrange("p nb (r q) s -> p nb r q s", q=2 * d)
            rb = vb[:].rearrange("p nb (r q) s -> p nb r q s", q=2 * d)
            # compare (i, i+d) for i = d + 2dt
            v.tensor_tensor(
                out=rb[:, :, 0:nruns, d:2 * d, :],
                in0=ra[:, :, 0:nruns, d:2 * d, :],
                in1=rb0_in1(ra, nruns, d),
                op=OP.max,
            )
            v.tensor_tensor(
                out=rb[:, :, 1:nruns + 1, 0:d, :],
                in0=ra[:, :, 0:nruns, d:2 * d, :],
                in1=ra[:, :, 1:nruns + 1, 0:d, :],
                op=OP.min,
            )
            # untouched boundary rows: [0, d) and [m-d, m)
            v.tensor_copy(vb[:, :, 0:d, :], va[:, :, 0:d, :])
            v.tensor_copy(vb[:, :, m - d:m, :], va[:, :, m - d:m, :])
        cur, other = other, cur
    return cur


def rb0_in1(ra, nruns, d):
    return ra[:, :, 1:nruns + 1, 0:d, :]


def _emit_tree_reduce(eng, OP, prod, scr, stage_slice, ns):
    """Sum over the P (rank) axis of prod [128, P, ns] via 7 binary-tree
    TT-add levels (fp16), final level fp32 into stage."""
    cur, other = prod, scr
    w = 64
    while w >= 2:
        eng.tensor_tensor(
            out=other[:, 0:w, :], in0=cur[:, 0:w, :],
            in1=cur[:, w:2 * w, :], op=OP.add,
        )
        cur, other = other, cur
        w //= 2
    eng.tensor_tensor(
        out=stage_slice, in0=cur[:, 0:1, :], in1=cur[:, 1:2, :], op=OP.add,
    )


def _build():
    global _BUILT
    if _BUILT is not None:
        return _BUILT
    from contextlib import ExitStack
    import concourse.bass as bass
    import concourse.bacc as bacc
    import concourse.tile as tile
    import concourse.mybir as mybir

    f32 = mybir.dt.float32
    f16 = mybir.dt.float16
    AF = mybir.ActivationFunctionType
    OP = mybir.AluOpType

    nc = bacc.Bacc("TRN2", target_bir_lowering=False, debug=False)

    xt_d = nc.dram_tensor("xt", [F, E * P], f16, kind="ExternalInput")
    w1_d = nc.dram_tensor("w1", [F, H], f16, kind="ExternalInput")
    w2_d = nc.dram_tensor("w2", [H, H], f16, kind="ExternalInput")
    w3_d = nc.dram_tensor("w3", [H, C], f16, kind="ExternalInput")
    b1_d = nc.dram_tensor("b1", [H, 1], f32, kind="ExternalInput")
    b2_d = nc.dram_tensor("b2", [H, 1], f32, kind="ExternalInput")
    # weight table, rank-major with slot-broadcast: [128, P, NG]
    wrepts_d = nc.dram_tensor("wrepts", [128, P, NG], f16, kind="ExternalInput")
    offs_d = nc.dram_tensor("offs", [128, 1], f32, kind="ExternalInput")
    epst_d = nc.dram_tensor("epst", [64, GALL], f32, kind="ExternalInput")

    mus_d = nc.dram_tensor("mus_t", [64, GALL], f32, kind="ExternalOutput")
    lv_d = nc.dram_tensor("logvars_t", [64, GALL], f32, kind="ExternalOutput")
    smp_d = nc.dram_tensor("samples_t", [64, GALL], f32, kind="ExternalOutput")

    with tile.TileContext(nc) as tc:
        with ExitStack() as ctx:
            consts = ctx.enter_context(tc.tile_pool(name="consts", bufs=1))
            xpool = ctx.enter_context(tc.tile_pool(name="x", bufs=2))
            hpool = ctx.enter_context(tc.tile_pool(name="h", bufs=3))
            zap = ctx.enter_context(tc.tile_pool(name="za", bufs=3))
            zbp = ctx.enter_context(tc.tile_pool(name="zb", bufs=3))
            prp = ctx.enter_context(tc.tile_pool(name="prod", bufs=2))
            spool = ctx.enter_context(tc.tile_pool(name="stage", bufs=1))
            epool = ctx.enter_context(tc.tile_pool(name="epi", bufs=1))
            ps1 = ctx.enter_context(tc.tile_pool(name="ps1", bufs=2, space="PSUM"))
            ps2 = ctx.enter_context(tc.tile_pool(name="ps2", bufs=1, space="PSUM"))
            ps3 = ctx.enter_context(tc.tile_pool(name="ps3", bufs=2, space="PSUM"))

            w1_s = consts.tile([F, H], f16)
            nc.sync.dma_start(out=w1_s[:], in_=w1_d[:])
            w2_s = consts.tile([H, H], f16)
            nc.sync.dma_start(out=w2_s[:], in_=w2_d[:])
            w3_s = consts.tile([H, C], f16)
            nc.sync.dma_start(out=w3_s[:], in_=w3_d[:])
            b1_s = consts.tile([H, 1], f32)
            nc.sync.dma_start(out=b1_s[:], in_=b1_d[:])
            b2_s = consts.tile([H, 1], f32)
            nc.sync.dma_start(out=b2_s[:], in_=b2_d[:])
            # wrepts (1 MB) is not needed until the first pooling ~60us in;
            # its dma_start is issued after chunk 0's input DMA so the first
            # MLP tile isn't queued behind it.
            wrepts_s = consts.tile([128, P, NG], f16)
            offs_s = consts.tile([128, 1], f32)

            stage = spool.tile([128, GALL], f32)

            chunks = _chunks(E)
            ev0 = 0
            for st, st_e in enumerate(chunks):
                ng = st_e // 4
                ngq = ng // 4
                col0 = ev0 // 4
                last = st == len(chunks) - 1
                ramp = st == 0
                xt_s = xpool.tile([F, st_e * P], f16, tag="xt")
                nc.sync.dma_start(
                    out=xt_s[:], in_=xt_d[:, ev0 * P:(ev0 + st_e) * P]
                )
                if st == 0:
                    nc.sync.dma_start(out=wrepts_s[:], in_=wrepts_d[:])
                    nc.sync.dma_start(out=offs_s[:], in_=offs_d[:])
                zA = zap.tile([128, P, ng], f16, tag="zA")
                zB = zbp.tile([128, P, ng], f16, tag="zB")

                for gq in range(ngq):
                    pz = ps3.tile([128, 4 * P], f32, tag="pz")
                    for half in range(2):
                        g2 = gq * 4 + half * 2        # first of 2 groups
                        ph1 = ps1.tile([128, 1024], f32, tag="ph1")
                        for gg in range(2):
                            nc.tensor.matmul(
                                ph1[:, gg * 512:(gg + 1) * 512], w1_s[:],
                                xt_s[:, (g2 + gg) * 512:(g2 + gg + 1) * 512],
                                start=True, stop=True,
                            )
                        h1 = hpool.tile([128, 1024], f16, tag="h1")
                        if ramp and half == 0:
                            nc.vector.tensor_scalar(
                                out=h1[:], in0=ph1[:], scalar1=b1_s[:],
                                scalar2=0.0, op0=OP.add, op1=OP.max)
                        else:
                            nc.scalar.activation(h1[:], ph1[:], AF.Relu,
                                                 bias=b1_s[:])
                        ph2 = ps2.tile([128, 1024], f32, tag="ph2")
                        for gg in range(2):
                            nc.tensor.matmul(
                                ph2[:, gg * 512:(gg + 1) * 512], w2_s[:],
                                h1[:, gg * 512:(gg + 1) * 512],
                                start=True, stop=True)
                        h2 = hpool.tile([128, 1024], f16, tag="h2")
                        if ramp and half == 1:
                            nc.vector.tensor_scalar(
                                out=h2[:], in0=ph2[:], scalar1=b2_s[:],
                                scalar2=0.0, op0=OP.add, op1=OP.max)
                        else:
                            nc.scalar.activation(h2[:], ph2[:], AF.Relu,
                                                 bias=b2_s[:])
                        for gg in range(2):
                            for e4 in range(4):
                                nc.tensor.matmul(
                                    pz[32 * e4:32 * (e4 + 1),
                                       (half * 2 + gg) * P:(half * 2 + gg + 1) * P],
                                    w3_s[:],
                                    h2[:, (gg * 4 + e4) * P:(gg * 4 + e4 + 1) * P],
                                    start=True, stop=True,
                                    tile_position=(0, 32 * e4),
                                )
                    nc.scalar.activation(
                        zA[:, :, 4 * gq:4 * (gq + 1)].rearrange("a p s -> a s p"),
                        pz[:].rearrange("a (s p) -> a s p", p=P),
                        AF.Copy,
                    )

                # ---- sort (DVE) ----
                cur = _emit_batcher(nc.vector, OP, zA, zB)

                # ---- rank-weighted pooling ----
                # GPSIMD normally; DVE for the last chunk (avoids draining
                # behind the slower engine at the end).
                eng = nc.vector if last else nc.gpsimd
                prod = prp.tile([128, P, ng], f16, tag="prod")
                eng.tensor_tensor(
                    out=prod[:], in0=cur[:], in1=wrepts_s[:, :, 0:ng],
                    op=OP.mult,
                )
                scr = prp.tile([128, 64, ng], f16, tag="scr")
                _emit_tree_reduce(
                    eng, OP, prod, scr,
                    stage[:, col0:col0 + ng].rearrange("a (o s) -> a o s", o=1),
                    ng,
                )
                ev0 += st_e

            # ---- epilogue ----
            # b3's pooled offset (channels host-permuted: within each
            # 32-partition block, 0:16 are mus, 16:32 logvars)
            nc.vector.tensor_scalar_add(stage[:], stage[:], offs_s[:])
            mus_al = epool.tile([64, GALL], f32)
            lv_al = epool.tile([64, GALL], f32)
            for q in range(4):
                nc.sync.dma_start(
                    out=mus_al[16 * q:16 * (q + 1), :],
                    in_=stage[32 * q:32 * q + 16, :],
                )
                nc.sync.dma_start(
                    out=lv_al[16 * q:16 * (q + 1), :],
                    in_=stage[32 * q + 16:32 * q + 32, :],
                )
            ex = epool.tile([64, GALL], f32)
            nc.scalar.activation(ex[:], lv_al[:], AF.Exp, scale=0.5)
            eps_s = epool.tile([64, GALL], f32)
            nc.sync.dma_start(out=eps_s[:], in_=epst_d[:])
            smp = epool.tile([64, GALL], f32)
            nc.vector.tensor_tensor(out=smp[:], in0=eps_s[:], in1=ex[:], op=OP.mult)
            nc.vector.tensor_tensor(out=smp[:], in0=smp[:], in1=mus_al[:], op=OP.add)
            nc.sync.dma_start(out=mus_d[:], in_=mus_al[:])
            nc.sync.dma_start(out=lv_d[:], in_=lv_al[:])
            nc.sync.dma_start(out=smp_d[:], in_=smp[:])

    nc.compile()
    _BUILT = nc
    return nc


def _host_prep(x, W1, b1, W2, b2, W3, b3, pool_weight, eps):
    x = np.asarray(x, np.float32)
    eps = np.asarray(eps, np.float32)
    W1 = np.asarray(W1, np.float32).astype(np.float16)
    W2 = np.asarray(W2, np.float32).astype(np.float16)
    W3 = np.asarray(W3, np.float32)
    b1 = np.asarray(b1, np.float32).reshape(H, 1)
    b2 = np.asarray(b2, np.float32).reshape(H, 1)
    b3 = np.asarray(b3, np.float32)
    pw = np.asarray(pool_weight, np.float32)

    # channel permutation: device channel c' maps to logical channel perm[c']
    # (mus channels 0,2,..,30 first, then logvar channels 1,3,..,31)
    perm = np.concatenate([np.arange(0, C, 2), np.arange(1, C, 2)])
    W3 = np.ascontiguousarray(W3[:, perm]).astype(np.float16)
    b3p = b3[perm]
    w_table = (pw @ _fspool_interp_matrix()).astype(np.float32)[perm]  # [32, 128]
    # pair-average ranks (1,2),(3,4),...,(125,126): the device skips the
    # final d=1 merge pass, leaving exactly these pairs possibly swapped.
    wp = w_table.copy()
    pav = 0.5 * (w_table[:, 1:127:2] + w_table[:, 2:127:2])
    wp[:, 1:127:2] = pav
    wp[:, 2:127:2] = pav
    w_table_dev = wp
    wrep = np.tile(w_table_dev, (4, 1))                                # [128, 128]
    wrepts = np.ascontiguousarray(
        np.broadcast_to(wrep[:, :, None], (128, P, NG))
    ).astype(np.float16)
    offs = np.tile(b3p * w_table.sum(axis=1), 4).reshape(128, 1).astype(np.float32)

    in_maps = []
    for c in range(NCORES):
        xs = x[c * E:(c + 1) * E]                                  # [E, 512]
        xt = np.ascontiguousarray(
            xs.reshape(E, P, F).transpose(2, 0, 1).reshape(F, E * P)
        ).astype(np.float16)
        es = eps[c * E:(c + 1) * E]                                # [E, 16]
        epst = np.ascontiguousarray(
            es.reshape(GALL, 4, LAT).transpose(1, 2, 0).reshape(64, GALL)
        )
        in_maps.append({
            "xt": xt, "w1": W1, "w2": W2, "w3": W3,
            "b1": b1, "b2": b2, "wrepts": wrepts,
            "offs": offs, "epst": epst,
        })
    return in_maps


def _host_post(results):
    mus = np.empty((B, LAT), np.float32)
    logvars = np.empty((B, LAT), np.float32)
    samples = np.empty((B, LAT), np.float32)
    for c, r in enumerate(results):
        for name, dst in (("mus_t", mus), ("logvars_t", logvars),
                          ("samples_t", samples)):
            t = r[name].reshape(4, LAT, GALL).transpose(2, 0, 1).reshape(E, LAT)
            dst[c * E:(c + 1) * E] = t
    return mus, logvars, samples


def kernel(**inputs):
    global LAST_RESULTS
    from concourse.bass_utils import run_bass_kernel_spmd

    nc = _build()
    in_maps = _host_prep(**inputs)
    trace = bool(int(os.environ.get("KERNEL_TRACE", "0")))
    res = run_bass_kernel_spmd(nc, in_maps, list(range(NCORES)), trace=trace)
    LAST_RESULTS = res
    return _host_post(res.results)


# revision 19
# speedup vs baseline: 1.0550x; 1.0550x over previous
"""Trainium2 Bass kernel for nn_Encoder (FSPool set encoder).

Computation per event b (8192 events, data-parallel over 8 cores):
  h = relu(x[b].reshape(128,4) @ W1 + b1)        # per-particle MLP
  h = relu(h @ W2 + b2)
  z = h @ W3 (+ b3)                              # [128 particles, 32 ch]
  z_sorted = sort_desc(z.T, axis=-1)             # per-channel sort over particles
  pooled[c] = sum_p z_sorted[c,p] * w[c,p]       # rank-weighted pool
  mus = pooled[::2]; logvars = pooled[1::2]
  samples = mus + eps * exp(0.5*logvars)

Device layout (per core, 1024 events, variable-size supertiles):
  - MLP on TensorE (hidden on partitions); relu+cast on ACT in 2-group
    (1024-col) PSUM tiles; z materialized channel-major via matmul3 with
    tile_position: partition = 32*(e%4)+c.
  - Per-row descending 128-sort: Batcher odd-even merge-sort (1471
    comparators vs bitonic's 1792) on the DVE in fp16, event "slots"
    packed along the inner free axis for 2x DVE mode. Inner merge passes
    touch only [d, m-d) of each block; untouched rows ping-pong via 4x
    tensor_copy.
  - Rank-weighted pooling (z_sorted*w then 7-level binary-tree sum over
    ranks) runs on GPSIMD, which has no other mid-kernel work; the last
    supertile's pooling runs on the DVE so the kernel doesn't drain
    waiting on the slower engine.
  - b3 never enters the sort: its pooled contribution is a per-partition
    offset added to the stage at the end.
"""

import os
import numpy as np

NCORES = 8
B = 8192
P = 128          # particles per event (set size)
F = 4            # input features per particle
H = 128          # hidden width
C = 32           # 2*LATENT pooled channels
LAT = 16
NPIECES = 20

E = B // NCORES          # events per core
ST_E = 128               # max events per supertile
NG = ST_E // 4           # max groups (slots) per supertile: 32
GALL = E // 4            # total groups per core (stage columns): 256

_BUILT = None
LAST_RESULTS = None      # test harness can inspect exec_time_ns / profile


def _chunks(e_total):
    """Event counts per super-tile: small head/tail tiles shorten the
    pipeline ramp-in and drain."""
    if e_total >= 8 * ST_E:
        q3 = 3 * ST_E // 4
        mid = (e_total - ST_E // 2 - 2 * q3) // ST_E
        return [ST_E // 2, q3] + [ST_E] * mid + [q3]
    out = []
    left = e_total
    while left > 0:
        c = min(ST_E, left)
        out.append(c)
        left -= c
    return out


def _fspool_interp_matrix():
    """M [21, 128] with w_table = pool_weight @ M (matches reference math)."""
    pos = (np.arange(P, dtype=np.float32) / np.float32(P - 1)) * np.float32(NPIECES)
    idx = np.clip(pos.astype(np.int32), 0, NPIECES)
    frac = pos - idx.astype(np.float32)
    M = np.zeros((NPIECES + 1, P), dtype=np.float32)
    for p in range(P):
        i = int(idx[p])
        M[i, p] += np.float32(1.0) - frac[p]
        M[min(i + 1, NPIECES), p] += frac[p]
    return M


def _batcher_passes(n=P, skip_final_d1=True):
    """(kind, m, d) pass list for Batcher odd-even mergesort of n.

    With skip_final_d1, the last cleanup pass of the final merge is
    dropped: the result is sorted except that pairs (2k+1, 2k+2) may be
    swapped. The pooling weight table is pair-averaged over exactly
    those pairs (host side), which makes the pooled sum invariant to
    the missing pass up to the (tiny) within-pair weight variation."""
    out = []
    m = 2
    while m <= n:
        out.append(("first", m, m // 2))
        d = m // 4
        while d >= 1:
            if not (skip_final_d1 and m == n and d == 1):
                out.append(("inner", m, d))
            d //= 2
        m *= 2
    return out


def _emit_batcher(v, OP, zA, zB, n=P):
    """Descending Batcher sort of the position axis of zA [128, P, s].
    Ping-pongs zA/zB every pass (28 passes, even -> result in zA)."""
    cur, other = zA, zB
    for kind, m, d in _batcher_passes(n):
        va = cur[:].rearrange("p (nb m) s -> p nb m s", m=m)
        vb = other[:].rearrange("p (nb m) s -> p nb m s", m=m)
        if kind == "first":
            v.tensor_tensor(out=vb[:, :, 0:d, :], in0=va[:, :, 0:d, :],
                            in1=va[:, :, d:m, :], op=OP.max)
            v.tensor_tensor(out=vb[:, :, d:m, :], in0=va[:, :, 0:d, :],
                            in1=va[:, :, d:m, :], op=OP.min)
        else:
            nruns = m // (2 * d) - 1
            # runs start at d + 2dt: view run axis explicitly
            ra = va[:].rearrange("p nb (r q) s -> p nb r q s", q=2 * d)
            rb = vb[:].rearrange("p nb (r q) s -> p nb r q s", q=2 * d)
            # compare (i, i+d) for i = d + 2dt
            v.tensor_tensor(
                out=rb[:, :, 0:nruns, d:2 * d, :],
                in0=ra[:, :, 0:nruns, d:2 * d, :],
                in1=rb0_in1(ra, nruns, d),
                op=OP.max,
            )
            v.tensor_tensor(
                out=rb[:, :, 1:nruns + 1, 0:d, :],
                in0=ra[:, :, 0:nruns, d:2 * d, :],
                in1=ra[:, :, 1:nruns + 1, 0:d, :],
                op=OP.min,
            )
            # Untouched boundary rows [0, d) and [m-d, m). Only the FIRST
            # inner pass of a merge level must copy them: its copy makes
            # the two ping-pong buffers agree on those rows, later passes
            # have strictly nested (and never again touched) boundaries, so
            # the stale buffer already holds the correct values.
            if d == m // 4:
                v.tensor_copy(vb[:, :, 0:d, :], va[:, :, 0:d, :])
                v.tensor_copy(vb[:, :, m - d:m, :], va[:, :, m - d:m, :])
        cur, other = other, cur
    return cur


def rb0_in1(ra, nruns, d):
    return ra[:, :, 1:nruns + 1, 0:d, :]


def _emit_tree_reduce(eng, OP, prod, scr, stage_slice, ns):
    """Sum over the P (rank) axis of prod [128, P, ns] via 7 binary-tree
    TT-add levels (fp16), final level fp32 into stage."""
    cur, other = prod, scr
    w = 64
    while w >= 2:
        eng.tensor_tensor(
            out=other[:, 0:w, :], in0=cur[:, 0:w, :],
            in1=cur[:, w:2 * w, :], op=OP.add,
        )
        cur, other = other, cur
        w //= 2
    eng.tensor_tensor(
        out=stage_slice, in0=cur[:, 0:1, :], in1=cur[:, 1:2, :], op=OP.add,
    )


def _build():
    global _BUILT
    if _BUILT is not None:
        return _BUILT
    from contextlib import ExitStack
    import concourse.bass as bass
    import concourse.bacc as bacc
    import concourse.tile as tile
    import concourse.mybir as mybir

    f32 = mybir.dt.float32
    f16 = mybir.dt.float16
    AF = mybir.ActivationFunctionType
    OP = mybir.AluOpType

    nc = bacc.Bacc("TRN2", target_bir_lowering=False, debug=False)

    xt_d = nc.dram_tensor("xt", [F, E * P], f16, kind="ExternalInput")
    w1_d = nc.dram_tensor("w1", [F, H], f16, kind="ExternalInput")
    w2_d = nc.dram_tensor("w2", [H, H], f16, kind="ExternalInput")
    w3_d = nc.dram_tensor("w3", [H, C], f16, kind="ExternalInput")
    b1_d = nc.dram_tensor("b1", [H, 1], f32, kind="ExternalInput")
    b2_d = nc.dram_tensor("b2", [H, 1], f32, kind="ExternalInput")
    # weight table, rank-major with slot-broadcast: [128, P, NG]
    wrepts_d = nc.dram_tensor("wrepts", [128, P, NG], f16, kind="ExternalInput")
    offs_d = nc.dram_tensor("offs", [128, 1], f32, kind="ExternalInput")
    epst_d = nc.dram_tensor("epst", [64, GALL], f32, kind="ExternalInput")

    mus_d = nc.dram_tensor("mus_t", [64, GALL], f32, kind="ExternalOutput")
    lv_d = nc.dram_tensor("logvars_t", [64, GALL], f32, kind="ExternalOutput")
    smp_d = nc.dram_tensor("samples_t", [64, GALL], f32, kind="ExternalOutput")

    with tile.TileContext(nc) as tc:
        with ExitStack() as ctx:
            consts = ctx.enter_context(tc.tile_pool(name="consts", bufs=1))
            xpool = ctx.enter_context(tc.tile_pool(name="x", bufs=2))
            hpool = ctx.enter_context(tc.tile_pool(name="h", bufs=3))
            zap = ctx.enter_context(tc.tile_pool(name="za", bufs=3))
            zbp = ctx.enter_context(tc.tile_pool(name="zb", bufs=3))
            prp = ctx.enter_context(tc.tile_pool(name="prod", bufs=2))
            spool = ctx.enter_context(tc.tile_pool(name="stage", bufs=1))
            epool = ctx.enter_context(tc.tile_pool(name="epi", bufs=1))
            ps1 = ctx.enter_context(tc.tile_pool(name="ps1", bufs=2, space="PSUM"))
            ps2 = ctx.enter_context(tc.tile_pool(name="ps2", bufs=1, space="PSUM"))
            ps3 = ctx.enter_context(tc.tile_pool(name="ps3", bufs=2, space="PSUM"))

            w1_s = consts.tile([F, H], f16)
            nc.sync.dma_start(out=w1_s[:], in_=w1_d[:])
            w2_s = consts.tile([H, H], f16)
            nc.sync.dma_start(out=w2_s[:], in_=w2_d[:])
            w3_s = consts.tile([H, C], f16)
            nc.sync.dma_start(out=w3_s[:], in_=w3_d[:])
            b1_s = consts.tile([H, 1], f32)
            nc.sync.dma_start(out=b1_s[:], in_=b1_d[:])
            b2_s = consts.tile([H, 1], f32)
            nc.sync.dma_start(out=b2_s[:], in_=b2_d[:])
            # wrepts (1 MB) is not needed until the first pooling ~60us in;
            # its dma_start is issued after chunk 0's input DMA so the first
            # MLP tile isn't queued behind it.
            wrepts_s = consts.tile([128, P, NG], f16)
            offs_s = consts.tile([128, 1], f32)

            stage = spool.tile([128, GALL], f32)

            chunks = _chunks(E)
            ev0 = 0
            for st, st_e in enumerate(chunks):
                ng = st_e // 4
                ngq = ng // 4
                col0 = ev0 // 4
                last = st == len(chunks) - 1
                ramp = st == 0
                xt_s = xpool.tile([F, st_e * P], f16, tag="xt")
                nc.sync.dma_start(
                    out=xt_s[:], in_=xt_d[:, ev0 * P:(ev0 + st_e) * P]
                )
                if st == 0:
                    nc.sync.dma_start(out=wrepts_s[:], in_=wrepts_d[:])
                    nc.sync.dma_start(out=offs_s[:], in_=offs_d[:])
                zA = zap.tile([128, P, ng], f16, tag="zA")
                zB = zbp.tile([128, P, ng], f16, tag="zB")

                for gq in range(ngq):
                    pz = ps3.tile([128, 4 * P], f32, tag="pz")
                    for half in range(2):
                        g2 = gq * 4 + half * 2        # first of 2 groups
                        ph1 = ps1.tile([128, 1024], f32, tag="ph1")
                        for gg in range(2):
                            nc.tensor.matmul(
                                ph1[:, gg * 512:(gg + 1) * 512], w1_s[:],
                                xt_s[:, (g2 + gg) * 512:(g2 + gg + 1) * 512],
                                start=True, stop=True,
                            )
                        h1 = hpool.tile([128, 1024], f16, tag="h1")
                        if ramp and half == 0:
                            nc.vector.tensor_scalar(
                                out=h1[:], in0=ph1[:], scalar1=b1_s[:],
                                scalar2=0.0, op0=OP.add, op1=OP.max)
                        else:
                            nc.scalar.activation(h1[:], ph1[:], AF.Relu,
                                                 bias=b1_s[:])
                        ph2 = ps2.tile([128, 1024], f32, tag="ph2")
                        for gg in range(2):
                            nc.tensor.matmul(
                                ph2[:, gg * 512:(gg + 1) * 512], w2_s[:],
                                h1[:, gg * 512:(gg + 1) * 512],
                                start=True, stop=True)
                        h2 = hpool.tile([128, 1024], f16, tag="h2")
                        if ramp and half == 1:
                            nc.vector.tensor_scalar(
                                out=h2[:], in0=ph2[:], scalar1=b2_s[:],
                                scalar2=0.0, op0=OP.add, op1=OP.max)
                        else:
                            nc.scalar.activation(h2[:], ph2[:], AF.Relu,
                                                 bias=b2_s[:])
                        for gg in range(2):
                            for e4 in range(4):
                                nc.tensor.matmul(
                                    pz[32 * e4:32 * (e4 + 1),
                                       (half * 2 + gg) * P:(half * 2 + gg + 1) * P],
                                    w3_s[:],
                                    h2[:, (gg * 4 + e4) * P:(gg * 4 + e4 + 1) * P],
                                    start=True, stop=True,
                                    tile_position=(0, 32 * e4),
                                )
                    nc.scalar.activation(
                        zA[:, :, 4 * gq:4 * (gq + 1)].rearrange("a p s -> a s p"),
                        pz[:].rearrange("a (s p) -> a s p", p=P),
                        AF.Copy,
                    )

                # ---- sort (DVE) ----
                cur = _emit_batcher(nc.vector, OP, zA, zB)

                # ---- rank-weighted pooling ----
                # GPSIMD normally; DVE for the last chunk (avoids draining
                # behind the slower engine at the end).
                eng = nc.vector if last else nc.gpsimd
                prod = prp.tile([128, P, ng], f16, tag="prod")
                eng.tensor_tensor(
                    out=prod[:], in0=cur[:], in1=wrepts_s[:, :, 0:ng],
                    op=OP.mult,
                )
                scr = prp.tile([128, 64, ng], f16, tag="scr")
                _emit_tree_reduce(
                    eng, OP, prod, scr,
                    stage[:, col0:col0 + ng].rearrange("a (o s) -> a o s", o=1),
                    ng,
                )
                ev0 += st_e

            # ---- epilogue ----
            # b3's pooled offset (channels host-permuted: within each
            # 32-partition block, 0:16 are mus, 16:32 logvars)
            nc.vector.tensor_scalar_add(stage[:], stage[:], offs_s[:])
            mus_al = epool.tile([64, GALL], f32)
            lv_al = epool.tile([64, GALL], f32)
            for q in range(4):
                nc.sync.dma_start(
                    out=mus_al[16 * q:16 * (q + 1), :],
                    in_=stage[32 * q:32 * q + 16, :],
                )
                nc.sync.dma_start(
                    out=lv_al[16 * q:16 * (q + 1), :],
                    in_=stage[32 * q + 16:32 * q + 32, :],
                )
            ex = epool.tile([64, GALL], f32)
            nc.scalar.activation(ex[:], lv_al[:], AF.Exp, scale=0.5)
            eps_s = epool.tile([64, GALL], f32)
            nc.sync.dma_start(out=eps_s[:], in_=epst_d[:])
            smp = epool.tile([64, GALL], f32)
            nc.vector.tensor_tensor(out=smp[:], in0=eps_s[:], in1=ex[:], op=OP.mult)
            nc.vector.tensor_tensor(out=smp[:], in0=smp[:], in1=mus_al[:], op=OP.add)
            nc.sync.dma_start(out=mus_d[:], in_=mus_al[:])
            nc.sync.dma_start(out=lv_d[:], in_=lv_al[:])
            nc.sync.dma_start(out=smp_d[:], in_=smp[:])

    nc.compile()
    _BUILT = nc
    return nc


def _host_prep(x, W1, b1, W2, b2, W3, b3, pool_weight, eps):
    x = np.asarray(x, np.float32)
    eps = np.asarray(eps, np.float32)
    W1 = np.asarray(W1, np.float32).astype(np.float16)
    W2 = np.asarray(W2, np.float32).astype(np.float16)
    W3 = np.asarray(W3, np.float32)
    b1 = np.asarray(b1, np.float32).reshape(H, 1)
    b2 = np.asarray(b2, np.float32).reshape(H, 1)
    b3 = np.asarray(b3, np.float32)
    pw = np.asarray(pool_weight, np.float32)

    # channel permutation: device channel c' maps to logical channel perm[c']
    # (mus channels 0,2,..,30 first, then logvar channels 1,3,..,31)
    perm = np.concatenate([np.arange(0, C, 2), np.arange(1, C, 2)])
    W3 = np.ascontiguousarray(W3[:, perm]).astype(np.float16)
    b3p = b3[perm]
    w_table = (pw @ _fspool_interp_matrix()).astype(np.float32)[perm]  # [32, 128]
    # pair-average ranks (1,2),(3,4),...,(125,126): the device skips the
    # final d=1 merge pass, leaving exactly these pairs possibly swapped.
    wp = w_table.copy()
    pav = 0.5 * (w_table[:, 1:127:2] + w_table[:, 2:127:2])
    wp[:, 1:127:2] = pav
    wp[:, 2:127:2] = pav
    w_table_dev = wp
    wrep = np.tile(w_table_dev, (4, 1))                                # [128, 128]
    wrepts = np.ascontiguousarray(
        np.broadcast_to(wrep[:, :, None], (128, P, NG))
    ).astype(np.float16)
    offs = np.tile(b3p * w_table.sum(axis=1), 4).reshape(128, 1).astype(np.float32)

    in_maps = []
    for c in range(NCORES):
        xs = x[c * E:(c + 1) * E]                                  # [E, 512]
        xt = np.ascontiguousarray(
            xs.reshape(E, P, F).transpose(2, 0, 1).reshape(F, E * P)
        ).astype(np.float16)
        es = eps[c * E:(c + 1) * E]                                # [E, 16]
        epst = np.ascontiguousarray(
            es.reshape(GALL, 4, LAT).transpose(1, 2, 0).reshape(64, GALL)
        )
        in_maps.append({
            "xt": xt, "w1": W1, "w2": W2, "w3": W3,
            "b1": b1, "b2": b2, "wrepts": wrepts,
            "offs": offs, "epst": epst,
        })
    return in_maps


def _host_post(results):
    mus = np.empty((B, LAT), np.float32)
    logvars = np.empty((B, LAT), np.float32)
    samples = np.empty((B, LAT), np.float32)
    for c, r in enumerate(results):
        for name, dst in (("mus_t", mus), ("logvars_t", logvars),
                          ("samples_t", samples)):
            t = r[name].reshape(4, LAT, GALL).transpose(2, 0, 1).reshape(E, LAT)
            dst[c * E:(c + 1) * E] = t
    return mus, logvars, samples


def kernel(**inputs):
    global LAST_RESULTS
    from concourse.bass_utils import run_bass_kernel_spmd

    nc = _build()
    in_maps = _host_prep(**inputs)
    trace = bool(int(os.environ.get("KERNEL_TRACE", "0")))
    res = run_bass_kernel_spmd(nc, in_maps, list(range(NCORES)), trace=trace)
    LAST_RESULTS = res
    return _host_post(res.results)


# revision 20
# speedup vs baseline: 1.0670x; 1.0113x over previous
"""Trainium2 Bass kernel for nn_Encoder (FSPool set encoder).

Computation per event b (8192 events, data-parallel over 8 cores):
  h = relu(x[b].reshape(128,4) @ W1 + b1)        # per-particle MLP
  h = relu(h @ W2 + b2)
  z = h @ W3 (+ b3)                              # [128 particles, 32 ch]
  z_sorted = sort_desc(z.T, axis=-1)             # per-channel sort over particles
  pooled[c] = sum_p z_sorted[c,p] * w[c,p]       # rank-weighted pool
  mus = pooled[::2]; logvars = pooled[1::2]
  samples = mus + eps * exp(0.5*logvars)

Device layout (per core, 1024 events, variable-size supertiles):
  - MLP on TensorE (hidden on partitions); relu+cast on ACT in 2-group
    (1024-col) PSUM tiles; z materialized channel-major via matmul3 with
    tile_position: partition = 32*(e%4)+c.
  - Per-row descending 128-sort: Batcher odd-even merge-sort (1471
    comparators vs bitonic's 1792) on the DVE in fp16, event "slots"
    packed along the inner free axis for 2x DVE mode. Inner merge passes
    touch only [d, m-d) of each block; untouched rows ping-pong via 4x
    tensor_copy.
  - Rank-weighted pooling (z_sorted*w then 7-level binary-tree sum over
    ranks) runs on GPSIMD, which has no other mid-kernel work; the last
    supertile's pooling runs on the DVE so the kernel doesn't drain
    waiting on the slower engine.
  - b3 never enters the sort: its pooled contribution is a per-partition
    offset added to the stage at the end.
"""

import os
import numpy as np

NCORES = 8
B = 8192
P = 128          # particles per event (set size)
F = 4            # input features per particle
H = 128          # hidden width
C = 32           # 2*LATENT pooled channels
LAT = 16
NPIECES = 20

E = B // NCORES          # events per core
ST_E = 128               # max events per supertile
NG = ST_E // 4           # max groups (slots) per supertile: 32
GALL = E // 4            # total groups per core (stage columns): 256

_BUILT = None
LAST_RESULTS = None      # test harness can inspect exec_time_ns / profile


def _chunks(e_total):
    """Event counts per super-tile: small head/tail tiles shorten the
    pipeline ramp-in and drain."""
    if e_total >= 8 * ST_E:
        q3 = 3 * ST_E // 4
        mid = (e_total - ST_E // 4 - ST_E // 2 - 2 * q3) // ST_E
        rem = e_total - ST_E // 4 - ST_E // 2 - 2 * q3 - mid * ST_E
        return [ST_E // 4, ST_E // 2, q3] + [ST_E] * mid + [rem, q3]
    out = []
    left = e_total
    while left > 0:
        c = min(ST_E, left)
        out.append(c)
        left -= c
    return out


def _fspool_interp_matrix():
    """M [21, 128] with w_table = pool_weight @ M (matches reference math)."""
    pos = (np.arange(P, dtype=np.float32) / np.float32(P - 1)) * np.float32(NPIECES)
    idx = np.clip(pos.astype(np.int32), 0, NPIECES)
    frac = pos - idx.astype(np.float32)
    M = np.zeros((NPIECES + 1, P), dtype=np.float32)
    for p in range(P):
        i = int(idx[p])
        M[i, p] += np.float32(1.0) - frac[p]
        M[min(i + 1, NPIECES), p] += frac[p]
    return M


def _batcher_passes(n=P, skip_final_d1=True):
    """(kind, m, d) pass list for Batcher odd-even mergesort of n.

    With skip_final_d1, the last cleanup pass of the final merge is
    dropped: the result is sorted except that pairs (2k+1, 2k+2) may be
    swapped. The pooling weight table is pair-averaged over exactly
    those pairs (host side), which makes the pooled sum invariant to
    the missing pass up to the (tiny) within-pair weight variation."""
    out = []
    m = 2
    while m <= n:
        out.append(("first", m, m // 2))
        d = m // 4
        while d >= 1:
            if not (skip_final_d1 and m == n and d == 1):
                out.append(("inner", m, d))
            d //= 2
        m *= 2
    return out


def _emit_batcher(v, OP, zA, zB, n=P):
    """Descending Batcher sort of the position axis of zA [128, P, s].
    Ping-pongs zA/zB every pass (28 passes, even -> result in zA)."""
    cur, other = zA, zB
    for kind, m, d in _batcher_passes(n):
        va = cur[:].rearrange("p (nb m) s -> p nb m s", m=m)
        vb = other[:].rearrange("p (nb m) s -> p nb m s", m=m)
        if kind == "first":
            v.tensor_tensor(out=vb[:, :, 0:d, :], in0=va[:, :, 0:d, :],
                            in1=va[:, :, d:m, :], op=OP.max)
            v.tensor_tensor(out=vb[:, :, d:m, :], in0=va[:, :, 0:d, :],
                            in1=va[:, :, d:m, :], op=OP.min)
        else:
            nruns = m // (2 * d) - 1
            # runs start at d + 2dt: view run axis explicitly
            ra = va[:].rearrange("p nb (r q) s -> p nb r q s", q=2 * d)
            rb = vb[:].rearrange("p nb (r q) s -> p nb r q s", q=2 * d)
            # compare (i, i+d) for i = d + 2dt
            v.tensor_tensor(
                out=rb[:, :, 0:nruns, d:2 * d, :],
                in0=ra[:, :, 0:nruns, d:2 * d, :],
                in1=rb0_in1(ra, nruns, d),
                op=OP.max,
            )
            v.tensor_tensor(
                out=rb[:, :, 1:nruns + 1, 0:d, :],
                in0=ra[:, :, 0:nruns, d:2 * d, :],
                in1=ra[:, :, 1:nruns + 1, 0:d, :],
                op=OP.min,
            )
            # Untouched boundary rows [0, d) and [m-d, m). Only the FIRST
            # inner pass of a merge level must copy them: its copy makes
            # the two ping-pong buffers agree on those rows, later passes
            # have strictly nested (and never again touched) boundaries, so
            # the stale buffer already holds the correct values.
            if d == m // 4:
                v.tensor_copy(vb[:, :, 0:d, :], va[:, :, 0:d, :])
                v.tensor_copy(vb[:, :, m - d:m, :], va[:, :, m - d:m, :])
        cur, other = other, cur
    return cur


def rb0_in1(ra, nruns, d):
    return ra[:, :, 1:nruns + 1, 0:d, :]


def _emit_tree_reduce(eng, OP, prod, scr, stage_slice, ns):
    """Sum over the P (rank) axis of prod [128, P, ns] via 7 binary-tree
    TT-add levels (fp16), final level fp32 into stage."""
    cur, other = prod, scr
    w = 64
    while w >= 2:
        eng.tensor_tensor(
            out=other[:, 0:w, :], in0=cur[:, 0:w, :],
            in1=cur[:, w:2 * w, :], op=OP.add,
        )
        cur, other = other, cur
        w //= 2
    eng.tensor_tensor(
        out=stage_slice, in0=cur[:, 0:1, :], in1=cur[:, 1:2, :], op=OP.add,
    )


def _build():
    global _BUILT
    if _BUILT is not None:
        return _BUILT
    from contextlib import ExitStack
    import concourse.bass as bass
    import concourse.bacc as bacc
    import concourse.tile as tile
    import concourse.mybir as mybir

    f32 = mybir.dt.float32
    f16 = mybir.dt.float16
    AF = mybir.ActivationFunctionType
    OP = mybir.AluOpType

    nc = bacc.Bacc("TRN2", target_bir_lowering=False, debug=False)

    xt_d = nc.dram_tensor("xt", [F, E * P], f16, kind="ExternalInput")
    w1_d = nc.dram_tensor("w1", [F, H], f16, kind="ExternalInput")
    w2_d = nc.dram_tensor("w2", [H, H], f16, kind="ExternalInput")
    w3_d = nc.dram_tensor("w3", [H, C], f16, kind="ExternalInput")
    b1_d = nc.dram_tensor("b1", [H, 1], f32, kind="ExternalInput")
    b2_d = nc.dram_tensor("b2", [H, 1], f32, kind="ExternalInput")
    # weight table, rank-major with slot-broadcast: [128, P, NG]
    wrepts_d = nc.dram_tensor("wrepts", [128, P, NG], f16, kind="ExternalInput")
    offs_d = nc.dram_tensor("offs", [128, 1], f32, kind="ExternalInput")
    epst_d = nc.dram_tensor("epst", [64, GALL], f32, kind="ExternalInput")

    mus_d = nc.dram_tensor("mus_t", [64, GALL], f32, kind="ExternalOutput")
    lv_d = nc.dram_tensor("logvars_t", [64, GALL], f32, kind="ExternalOutput")
    smp_d = nc.dram_tensor("samples_t", [64, GALL], f32, kind="ExternalOutput")

    with tile.TileContext(nc) as tc:
        with ExitStack() as ctx:
            consts = ctx.enter_context(tc.tile_pool(name="consts", bufs=1))
            xpool = ctx.enter_context(tc.tile_pool(name="x", bufs=2))
            hpool = ctx.enter_context(tc.tile_pool(name="h", bufs=3))
            zap = ctx.enter_context(tc.tile_pool(name="za", bufs=3))
            zbp = ctx.enter_context(tc.tile_pool(name="zb", bufs=3))
            prp = ctx.enter_context(tc.tile_pool(name="prod", bufs=2))
            spool = ctx.enter_context(tc.tile_pool(name="stage", bufs=1))
            epool = ctx.enter_context(tc.tile_pool(name="epi", bufs=1))
            ps1 = ctx.enter_context(tc.tile_pool(name="ps1", bufs=2, space="PSUM"))
            ps2 = ctx.enter_context(tc.tile_pool(name="ps2", bufs=1, space="PSUM"))
            ps3 = ctx.enter_context(tc.tile_pool(name="ps3", bufs=2, space="PSUM"))

            w1_s = consts.tile([F, H], f16)
            nc.sync.dma_start(out=w1_s[:], in_=w1_d[:])
            w2_s = consts.tile([H, H], f16)
            nc.sync.dma_start(out=w2_s[:], in_=w2_d[:])
            w3_s = consts.tile([H, C], f16)
            nc.sync.dma_start(out=w3_s[:], in_=w3_d[:])
            b1_s = consts.tile([H, 1], f32)
            nc.sync.dma_start(out=b1_s[:], in_=b1_d[:])
            b2_s = consts.tile([H, 1], f32)
            nc.sync.dma_start(out=b2_s[:], in_=b2_d[:])
            # wrepts (1 MB) is not needed until the first pooling ~60us in;
            # its dma_start is issued after chunk 0's input DMA so the first
            # MLP tile isn't queued behind it.
            wrepts_s = consts.tile([128, P, NG], f16)
            offs_s = consts.tile([128, 1], f32)

            stage = spool.tile([128, GALL], f32)

            chunks = _chunks(E)
            ev0 = 0
            for st, st_e in enumerate(chunks):
                ng = st_e // 4
                ngq = ng // 4
                col0 = ev0 // 4
                last = st == len(chunks) - 1
                ramp = st == 0
                xt_s = xpool.tile([F, st_e * P], f16, tag="xt")
                nc.sync.dma_start(
                    out=xt_s[:], in_=xt_d[:, ev0 * P:(ev0 + st_e) * P]
                )
                if st == 0:
                    nc.sync.dma_start(out=wrepts_s[:], in_=wrepts_d[:])
                    nc.sync.dma_start(out=offs_s[:], in_=offs_d[:])
                zA = zap.tile([128, P, ng], f16, tag="zA")
                zB = zbp.tile([128, P, ng], f16, tag="zB")

                for gq in range(ngq):
                    pz = ps3.tile([128, 4 * P], f32, tag="pz")
                    for half in range(2):
                        g2 = gq * 4 + half * 2        # first of 2 groups
                        ph1 = ps1.tile([128, 1024], f32, tag="ph1")
                        for gg in range(2):
                            nc.tensor.matmul(
                                ph1[:, gg * 512:(gg + 1) * 512], w1_s[:],
                                xt_s[:, (g2 + gg) * 512:(g2 + gg + 1) * 512],
                                start=True, stop=True,
                            )
                        h1 = hpool.tile([128, 1024], f16, tag="h1")
                        if ramp and half == 0:
                            nc.vector.tensor_scalar(
                                out=h1[:], in0=ph1[:], scalar1=b1_s[:],
                                scalar2=0.0, op0=OP.add, op1=OP.max)
                        else:
                            nc.scalar.activation(h1[:], ph1[:], AF.Relu,
                                                 bias=b1_s[:])
                        ph2 = ps2.tile([128, 1024], f32, tag="ph2")
                        for gg in range(2):
                            nc.tensor.matmul(
                                ph2[:, gg * 512:(gg + 1) * 512], w2_s[:],
                                h1[:, gg * 512:(gg + 1) * 512],
                                start=True, stop=True)
                        h2 = hpool.tile([128, 1024], f16, tag="h2")
                        if ramp and half == 1:
                            nc.vector.tensor_scalar(
                                out=h2[:], in0=ph2[:], scalar1=b2_s[:],
                                scalar2=0.0, op0=OP.add, op1=OP.max)
                        else:
                            nc.scalar.activation(h2[:], ph2[:], AF.Relu,
                                                 bias=b2_s[:])
                        for gg in range(2):
                            for e4 in range(4):
                                nc.tensor.matmul(
                                    pz[32 * e4:32 * (e4 + 1),
                                       (half * 2 + gg) * P:(half * 2 + gg + 1) * P],
                                    w3_s[:],
                                    h2[:, (gg * 4 + e4) * P:(gg * 4 + e4 + 1) * P],
                                    start=True, stop=True,
                                    tile_position=(0, 32 * e4),
                                )
                    nc.scalar.activation(
                        zA[:, :, 4 * gq:4 * (gq + 1)].rearrange("a p s -> a s p"),
                        pz[:].rearrange("a (s p) -> a s p", p=P),
                        AF.Copy,
                    )

                # ---- sort (DVE) ----
                cur = _emit_batcher(nc.vector, OP, zA, zB)

                # ---- rank-weighted pooling ----
                # GPSIMD normally; DVE for the last chunk (avoids draining
                # behind the slower engine at the end).
                eng = nc.vector if last else nc.gpsimd
                prod = prp.tile([128, P, ng], f16, tag="prod")
                eng.tensor_tensor(
                    out=prod[:], in0=cur[:], in1=wrepts_s[:, :, 0:ng],
                    op=OP.mult,
                )
                scr = prp.tile([128, 64, ng], f16, tag="scr")
                _emit_tree_reduce(
                    eng, OP, prod, scr,
                    stage[:, col0:col0 + ng].rearrange("a (o s) -> a o s", o=1),
                    ng,
                )
                ev0 += st_e

            # ---- epilogue ----
            # b3's pooled offset (channels host-permuted: within each
            # 32-partition block, 0:16 are mus, 16:32 logvars)
            nc.vector.tensor_scalar_add(stage[:], stage[:], offs_s[:])
            mus_al = epool.tile([64, GALL], f32)
            lv_al = epool.tile([64, GALL], f32)
            for q in range(4):
                nc.sync.dma_start(
                    out=mus_al[16 * q:16 * (q + 1), :],
                    in_=stage[32 * q:32 * q + 16, :],
                )
                nc.sync.dma_start(
                    out=lv_al[16 * q:16 * (q + 1), :],
                    in_=stage[32 * q + 16:32 * q + 32, :],
                )
            ex = epool.tile([64, GALL], f32)
            nc.scalar.activation(ex[:], lv_al[:], AF.Exp, scale=0.5)
            eps_s = epool.tile([64, GALL], f32)
            nc.sync.dma_start(out=eps_s[:], in_=epst_d[:])
            smp = epool.tile([64, GALL], f32)
            nc.vector.tensor_tensor(out=smp[:], in0=eps_s[:], in1=ex[:], op=OP.mult)
            nc.vector.tensor_tensor(out=smp[:], in0=smp[:], in1=mus_al[:], op=OP.add)
            nc.sync.dma_start(out=mus_d[:], in_=mus_al[:])
            nc.sync.dma_start(out=lv_d[:], in_=lv_al[:])
            nc.sync.dma_start(out=smp_d[:], in_=smp[:])

    nc.compile()
    _BUILT = nc
    return nc


def _host_prep(x, W1, b1, W2, b2, W3, b3, pool_weight, eps):
    x = np.asarray(x, np.float32)
    eps = np.asarray(eps, np.float32)
    W1 = np.asarray(W1, np.float32).astype(np.float16)
    W2 = np.asarray(W2, np.float32).astype(np.float16)
    W3 = np.asarray(W3, np.float32)
    b1 = np.asarray(b1, np.float32).reshape(H, 1)
    b2 = np.asarray(b2, np.float32).reshape(H, 1)
    b3 = np.asarray(b3, np.float32)
    pw = np.asarray(pool_weight, np.float32)

    # channel permutation: device channel c' maps to logical channel perm[c']
    # (mus channels 0,2,..,30 first, then logvar channels 1,3,..,31)
    perm = np.concatenate([np.arange(0, C, 2), np.arange(1, C, 2)])
    W3 = np.ascontiguousarray(W3[:, perm]).astype(np.float16)
    b3p = b3[perm]
    w_table = (pw @ _fspool_interp_matrix()).astype(np.float32)[perm]  # [32, 128]
    # pair-average ranks (1,2),(3,4),...,(125,126): the device skips the
    # final d=1 merge pass, leaving exactly these pairs possibly swapped.
    wp = w_table.copy()
    pav = 0.5 * (w_table[:, 1:127:2] + w_table[:, 2:127:2])
    wp[:, 1:127:2] = pav
    wp[:, 2:127:2] = pav
    w_table_dev = wp
    wrep = np.tile(w_table_dev, (4, 1))                                # [128, 128]
    wrepts = np.ascontiguousarray(
        np.broadcast_to(wrep[:, :, None], (128, P, NG))
    ).astype(np.float16)
    offs = np.tile(b3p * w_table.sum(axis=1), 4).reshape(128, 1).astype(np.float32)

    in_maps = []
    for c in range(NCORES):
        xs = x[c * E:(c + 1) * E]                                  # [E, 512]
        xt = np.ascontiguousarray(
            xs.reshape(E, P, F).transpose(2, 0, 1).reshape(F, E * P)
        ).astype(np.float16)
        es = eps[c * E:(c + 1) * E]                                # [E, 16]
        epst = np.ascontiguousarray(
            es.reshape(GALL, 4, LAT).transpose(1, 2, 0).reshape(64, GALL)
        )
        in_maps.append({
            "xt": xt, "w1": W1, "w2": W2, "w3": W3,
            "b1": b1, "b2": b2, "wrepts": wrepts,
            "offs": offs, "epst": epst,
        })
    return in_maps


def _host_post(results):
    mus = np.empty((B, LAT), np.float32)
    logvars = np.empty((B, LAT), np.float32)
    samples = np.empty((B, LAT), np.float32)
    for c, r in enumerate(results):
        for name, dst in (("mus_t", mus), ("logvars_t", logvars),
                          ("samples_t", samples)):
            t = r[name].reshape(4, LAT, GALL).transpose(2, 0, 1).reshape(E, LAT)
            dst[c * E:(c + 1) * E] = t
    return mus, logvars, samples


def kernel(**inputs):
    global LAST_RESULTS
    from concourse.bass_utils import run_bass_kernel_spmd

    nc = _build()
    in_maps = _host_prep(**inputs)
    trace = bool(int(os.environ.get("KERNEL_TRACE", "0")))
    res = run_bass_kernel_spmd(nc, in_maps, list(range(NCORES)), trace=trace)
    LAST_RESULTS = res
    return _host_post(res.results)


# revision 23
# speedup vs baseline: 1.0675x; 1.0005x over previous
"""Trainium2 Bass kernel for nn_Encoder (FSPool set encoder).

Computation per event b (8192 events, data-parallel over 8 cores):
  h = relu(x[b].reshape(128,4) @ W1 + b1)        # per-particle MLP
  h = relu(h @ W2 + b2)
  z = h @ W3 (+ b3)                              # [128 particles, 32 ch]
  z_sorted = sort_desc(z.T, axis=-1)             # per-channel sort over particles
  pooled[c] = sum_p z_sorted[c,p] * w[c,p]       # rank-weighted pool
  mus = pooled[::2]; logvars = pooled[1::2]
  samples = mus + eps * exp(0.5*logvars)

Device layout (per core, 1024 events, variable-size supertiles):
  - MLP on TensorE (hidden on partitions); relu+cast on ACT in 2-group
    (1024-col) PSUM tiles; z materialized channel-major via matmul3 with
    tile_position: partition = 32*(e%4)+c.
  - Per-row descending 128-sort: Batcher odd-even merge-sort (1471
    comparators vs bitonic's 1792) on the DVE in fp16, event "slots"
    packed along the inner free axis for 2x DVE mode. Inner merge passes
    touch only [d, m-d) of each block; untouched rows ping-pong via 4x
    tensor_copy.
  - Rank-weighted pooling (z_sorted*w then 7-level binary-tree sum over
    ranks) runs on GPSIMD, which has no other mid-kernel work; the last
    supertile's pooling runs on the DVE so the kernel doesn't drain
    waiting on the slower engine.
  - b3 never enters the sort: its pooled contribution is a per-partition
    offset added to the stage at the end.
"""

import os
import numpy as np

NCORES = 8
B = 8192
P = 128          # particles per event (set size)
F = 4            # input features per particle
H = 128          # hidden width
C = 32           # 2*LATENT pooled channels
LAT = 16
NPIECES = 20

E = B // NCORES          # events per core
ST_E = 128               # max events per supertile
NG = ST_E // 4           # max groups (slots) per supertile: 32
GALL = E // 4            # total groups per core (stage columns): 256

_BUILT = None
LAST_RESULTS = None      # test harness can inspect exec_time_ns / profile


def _chunks(e_total):
    """Event counts per super-tile: small head/tail tiles shorten the
    pipeline ramp-in and drain."""
    if e_total >= 8 * ST_E:
        q3 = 3 * ST_E // 4
        mid = (e_total - ST_E // 4 - ST_E // 2 - 2 * q3) // ST_E
        rem = e_total - ST_E // 4 - ST_E // 2 - 2 * q3 - mid * ST_E
        return [ST_E // 4, ST_E // 2, q3] + [ST_E] * mid + [rem, q3]
    out = []
    left = e_total
    while left > 0:
        c = min(ST_E, left)
        out.append(c)
        left -= c
    return out


def _fspool_interp_matrix():
    """M [21, 128] with w_table = pool_weight @ M (matches reference math)."""
    pos = (np.arange(P, dtype=np.float32) / np.float32(P - 1)) * np.float32(NPIECES)
    idx = np.clip(pos.astype(np.int32), 0, NPIECES)
    frac = pos - idx.astype(np.float32)
    M = np.zeros((NPIECES + 1, P), dtype=np.float32)
    for p in range(P):
        i = int(idx[p])
        M[i, p] += np.float32(1.0) - frac[p]
        M[min(i + 1, NPIECES), p] += frac[p]
    return M


def _batcher_passes(n=P, skip_final_d1=True):
    """(kind, m, d) pass list for Batcher odd-even mergesort of n.

    With skip_final_d1, the last cleanup pass of the final merge is
    dropped: the result is sorted except that pairs (2k+1, 2k+2) may be
    swapped. The pooling weight table is pair-averaged over exactly
    those pairs (host side), which makes the pooled sum invariant to
    the missing pass up to the (tiny) within-pair weight variation."""
    out = []
    m = 2
    while m <= n:
        out.append(("first", m, m // 2))
        d = m // 4
        while d >= 1:
            if not (skip_final_d1 and m == n and d == 1):
                out.append(("inner", m, d))
            d //= 2
        m *= 2
    return out


def _emit_batcher(v, OP, zA, zB, n=P):
    """Descending Batcher sort of the position axis of zA [128, P, s].
    Ping-pongs zA/zB every pass (28 passes, even -> result in zA)."""
    cur, other = zA, zB
    for kind, m, d in _batcher_passes(n):
        va = cur[:].rearrange("p (nb m) s -> p nb m s", m=m)
        vb = other[:].rearrange("p (nb m) s -> p nb m s", m=m)
        if kind == "first":
            v.tensor_tensor(out=vb[:, :, 0:d, :], in0=va[:, :, 0:d, :],
                            in1=va[:, :, d:m, :], op=OP.max)
            v.tensor_tensor(out=vb[:, :, d:m, :], in0=va[:, :, 0:d, :],
                            in1=va[:, :, d:m, :], op=OP.min)
        else:
            nruns = m // (2 * d) - 1
            # runs start at d + 2dt: view run axis explicitly
            ra = va[:].rearrange("p nb (r q) s -> p nb r q s", q=2 * d)
            rb = vb[:].rearrange("p nb (r q) s -> p nb r q s", q=2 * d)
            # compare (i, i+d) for i = d + 2dt
            v.tensor_tensor(
                out=rb[:, :, 0:nruns, d:2 * d, :],
                in0=ra[:, :, 0:nruns, d:2 * d, :],
                in1=rb0_in1(ra, nruns, d),
                op=OP.max,
            )
            v.tensor_tensor(
                out=rb[:, :, 1:nruns + 1, 0:d, :],
                in0=ra[:, :, 0:nruns, d:2 * d, :],
                in1=ra[:, :, 1:nruns + 1, 0:d, :],
                op=OP.min,
            )
            # Untouched boundary rows [0, d) and [m-d, m). Only the FIRST
            # inner pass of a merge level must copy them: its copy makes
            # the two ping-pong buffers agree on those rows, later passes
            # have strictly nested (and never again touched) boundaries, so
            # the stale buffer already holds the correct values.
            if d == m // 4:
                v.tensor_copy(vb[:, :, 0:d, :], va[:, :, 0:d, :])
                v.tensor_copy(vb[:, :, m - d:m, :], va[:, :, m - d:m, :])
        cur, other = other, cur
    return cur


def rb0_in1(ra, nruns, d):
    return ra[:, :, 1:nruns + 1, 0:d, :]


def _emit_tree_reduce(eng, OP, prod, scr, stage_slice, ns):
    """Sum over the P (rank) axis of prod [128, P, ns] via 7 binary-tree
    TT-add levels (fp16), final level fp32 into stage."""
    cur, other = prod, scr
    w = 64
    while w >= 2:
        eng.tensor_tensor(
            out=other[:, 0:w, :], in0=cur[:, 0:w, :],
            in1=cur[:, w:2 * w, :], op=OP.add,
        )
        cur, other = other, cur
        w //= 2
    eng.tensor_tensor(
        out=stage_slice, in0=cur[:, 0:1, :], in1=cur[:, 1:2, :], op=OP.add,
    )


def _build():
    global _BUILT
    if _BUILT is not None:
        return _BUILT
    from contextlib import ExitStack
    import concourse.bass as bass
    import concourse.bacc as bacc
    import concourse.tile as tile
    import concourse.mybir as mybir

    f32 = mybir.dt.float32
    f16 = mybir.dt.float16
    AF = mybir.ActivationFunctionType
    OP = mybir.AluOpType

    nc = bacc.Bacc("TRN2", target_bir_lowering=False, debug=False)

    xt_d = nc.dram_tensor("xt", [F, E * P], f16, kind="ExternalInput")
    w1_d = nc.dram_tensor("w1", [F, H], f16, kind="ExternalInput")
    w2_d = nc.dram_tensor("w2", [H, H], f16, kind="ExternalInput")
    w3_d = nc.dram_tensor("w3", [H, C], f16, kind="ExternalInput")
    b1_d = nc.dram_tensor("b1", [H, 1], f32, kind="ExternalInput")
    b2_d = nc.dram_tensor("b2", [H, 1], f32, kind="ExternalInput")
    # weight table, rank-major with slot-broadcast: [128, P, NG]
    wrepts_d = nc.dram_tensor("wrepts", [128, P, NG], f16, kind="ExternalInput")
    offs_d = nc.dram_tensor("offs", [128, 1], f32, kind="ExternalInput")
    epst_d = nc.dram_tensor("epst", [64, GALL], f32, kind="ExternalInput")

    mus_d = nc.dram_tensor("mus_t", [64, GALL], f32, kind="ExternalOutput")
    lv_d = nc.dram_tensor("logvars_t", [64, GALL], f32, kind="ExternalOutput")
    smp_d = nc.dram_tensor("samples_t", [64, GALL], f32, kind="ExternalOutput")

    with tile.TileContext(nc) as tc:
        with ExitStack() as ctx:
            consts = ctx.enter_context(tc.tile_pool(name="consts", bufs=1))
            xpool = ctx.enter_context(tc.tile_pool(name="x", bufs=2))
            hpool = ctx.enter_context(tc.tile_pool(name="h", bufs=3))
            zap = ctx.enter_context(tc.tile_pool(name="za", bufs=3))
            zbp = ctx.enter_context(tc.tile_pool(name="zb", bufs=3))
            prp = ctx.enter_context(tc.tile_pool(name="prod", bufs=2))
            spool = ctx.enter_context(tc.tile_pool(name="stage", bufs=1))
            epool = ctx.enter_context(tc.tile_pool(name="epi", bufs=1))
            ps1 = ctx.enter_context(tc.tile_pool(name="ps1", bufs=2, space="PSUM"))
            ps2 = ctx.enter_context(tc.tile_pool(name="ps2", bufs=1, space="PSUM"))
            ps3 = ctx.enter_context(tc.tile_pool(name="ps3", bufs=2, space="PSUM"))

            w1_s = consts.tile([F, H], f16)
            nc.sync.dma_start(out=w1_s[:], in_=w1_d[:])
            w2_s = consts.tile([H, H], f16)
            nc.sync.dma_start(out=w2_s[:], in_=w2_d[:])
            w3_s = consts.tile([H, C], f16)
            nc.sync.dma_start(out=w3_s[:], in_=w3_d[:])
            b1_s = consts.tile([H, 1], f32)
            nc.sync.dma_start(out=b1_s[:], in_=b1_d[:])
            b2_s = consts.tile([H, 1], f32)
            nc.sync.dma_start(out=b2_s[:], in_=b2_d[:])
            # wrepts (1 MB) is not needed until the first pooling ~60us in;
            # its dma_start is issued after chunk 0's input DMA so the first
            # MLP tile isn't queued behind it.
            wrepts_s = consts.tile([128, P, NG], f16)
            offs_s = consts.tile([128, 1], f32)

            stage = spool.tile([128, GALL], f32)
            # assembled per chunk: channels host-permuted so that within
            # each 32-partition stage block, 0:16 are mus, 16:32 logvars
            mus_al = epool.tile([64, GALL], f32)
            lv_al = epool.tile([64, GALL], f32)

            chunks = _chunks(E)
            ev0 = 0
            for st, st_e in enumerate(chunks):
                ng = st_e // 4
                ngq = ng // 4
                col0 = ev0 // 4
                last = st == len(chunks) - 1
                ramp = st == 0
                xt_s = xpool.tile([F, st_e * P], f16, tag="xt")
                nc.sync.dma_start(
                    out=xt_s[:], in_=xt_d[:, ev0 * P:(ev0 + st_e) * P]
                )
                if st == 0:
                    nc.sync.dma_start(out=wrepts_s[:], in_=wrepts_d[:])
                    nc.sync.dma_start(out=offs_s[:], in_=offs_d[:])
                zA = zap.tile([128, P, ng], f16, tag="zA")
                zB = zbp.tile([128, P, ng], f16, tag="zB")

                for gq in range(ngq):
                    pz = ps3.tile([128, 4 * P], f32, tag="pz")
                    for half in range(2):
                        g2 = gq * 4 + half * 2        # first of 2 groups
                        ph1 = ps1.tile([128, 1024], f32, tag="ph1")
                        for gg in range(2):
                            nc.tensor.matmul(
                                ph1[:, gg * 512:(gg + 1) * 512], w1_s[:],
                                xt_s[:, (g2 + gg) * 512:(g2 + gg + 1) * 512],
                                start=True, stop=True,
                            )
                        h1 = hpool.tile([128, 1024], f16, tag="h1")
                        if ramp and half == 0:
                            nc.vector.tensor_scalar(
                                out=h1[:], in0=ph1[:], scalar1=b1_s[:],
                                scalar2=0.0, op0=OP.add, op1=OP.max)
                        else:
                            nc.scalar.activation(h1[:], ph1[:], AF.Relu,
                                                 bias=b1_s[:])
                        ph2 = ps2.tile([128, 1024], f32, tag="ph2")
                        for gg in range(2):
                            nc.tensor.matmul(
                                ph2[:, gg * 512:(gg + 1) * 512], w2_s[:],
                                h1[:, gg * 512:(gg + 1) * 512],
                                start=True, stop=True)
                        h2 = hpool.tile([128, 1024], f16, tag="h2")
                        if ramp and half == 1:
                            nc.vector.tensor_scalar(
                                out=h2[:], in0=ph2[:], scalar1=b2_s[:],
                                scalar2=0.0, op0=OP.add, op1=OP.max)
                        else:
                            nc.scalar.activation(h2[:], ph2[:], AF.Relu,
                                                 bias=b2_s[:])
                        for gg in range(2):
                            for e4 in range(4):
                                nc.tensor.matmul(
                                    pz[32 * e4:32 * (e4 + 1),
                                       (half * 2 + gg) * P:(half * 2 + gg + 1) * P],
                                    w3_s[:],
                                    h2[:, (gg * 4 + e4) * P:(gg * 4 + e4 + 1) * P],
                                    start=True, stop=True,
                                    tile_position=(0, 32 * e4),
                                )
                    nc.scalar.activation(
                        zA[:, :, 4 * gq:4 * (gq + 1)].rearrange("a p s -> a s p"),
                        pz[:].rearrange("a (s p) -> a s p", p=P),
                        AF.Copy,
                    )

                # ---- sort (DVE) ----
                cur = _emit_batcher(nc.vector, OP, zA, zB)

                # ---- rank-weighted pooling ----
                # GPSIMD normally; DVE for the last chunk (avoids draining
                # behind the slower engine at the end).
                eng = nc.vector if last else nc.gpsimd
                prod = prp.tile([128, P, ng], f16, tag="prod")
                eng.tensor_tensor(
                    out=prod[:], in0=cur[:], in1=wrepts_s[:, :, 0:ng],
                    op=OP.mult,
                )
                scr = prp.tile([128, 64, ng], f16, tag="scr")
                _emit_tree_reduce(
                    eng, OP, prod, scr,
                    stage[:, col0:col0 + ng].rearrange("a (o s) -> a o s", o=1),
                    ng,
                )
                # b3's pooled offset + mus/lv assembly, pipelined per chunk
                # so the epilogue isn't one serial chain after the last sort
                eng.tensor_scalar_add(stage[:, col0:col0 + ng],
                                      stage[:, col0:col0 + ng], offs_s[:])
                for q in range(4):
                    nc.sync.dma_start(
                        out=mus_al[16 * q:16 * (q + 1), col0:col0 + ng],
                        in_=stage[32 * q:32 * q + 16, col0:col0 + ng],
                    )
                    nc.sync.dma_start(
                        out=lv_al[16 * q:16 * (q + 1), col0:col0 + ng],
                        in_=stage[32 * q + 16:32 * q + 32, col0:col0 + ng],
                    )
                ev0 += st_e

            # ---- epilogue (mus_al/lv_al assembled per chunk above) ----
            ex = epool.tile([64, GALL], f32)
            nc.scalar.activation(ex[:], lv_al[:], AF.Exp, scale=0.5)
            eps_s = epool.tile([64, GALL], f32)
            nc.sync.dma_start(out=eps_s[:], in_=epst_d[:])
            smp = epool.tile([64, GALL], f32)
            nc.vector.tensor_tensor(out=smp[:], in0=eps_s[:], in1=ex[:], op=OP.mult)
            nc.vector.tensor_tensor(out=smp[:], in0=smp[:], in1=mus_al[:], op=OP.add)
            nc.sync.dma_start(out=mus_d[:], in_=mus_al[:])
            nc.sync.dma_start(out=lv_d[:], in_=lv_al[:])
            nc.sync.dma_start(out=smp_d[:], in_=smp[:])

    nc.compile()
    _BUILT = nc
    return nc


def _host_prep(x, W1, b1, W2, b2, W3, b3, pool_weight, eps):
    x = np.asarray(x, np.float32)
    eps = np.asarray(eps, np.float32)
    W1 = np.asarray(W1, np.float32).astype(np.float16)
    W2 = np.asarray(W2, np.float32).astype(np.float16)
    W3 = np.asarray(W3, np.float32)
    b1 = np.asarray(b1, np.float32).reshape(H, 1)
    b2 = np.asarray(b2, np.float32).reshape(H, 1)
    b3 = np.asarray(b3, np.float32)
    pw = np.asarray(pool_weight, np.float32)

    # channel permutation: device channel c' maps to logical channel perm[c']
    # (mus channels 0,2,..,30 first, then logvar channels 1,3,..,31)
    perm = np.concatenate([np.arange(0, C, 2), np.arange(1, C, 2)])
    W3 = np.ascontiguousarray(W3[:, perm]).astype(np.float16)
    b3p = b3[perm]
    w_table = (pw @ _fspool_interp_matrix()).astype(np.float32)[perm]  # [32, 128]
    # pair-average ranks (1,2),(3,4),...,(125,126): the device skips the
    # final d=1 merge pass, leaving exactly these pairs possibly swapped.
    wp = w_table.copy()
    pav = 0.5 * (w_table[:, 1:127:2] + w_table[:, 2:127:2])
    wp[:, 1:127:2] = pav
    wp[:, 2:127:2] = pav
    w_table_dev = wp
    wrep = np.tile(w_table_dev, (4, 1))                                # [128, 128]
    wrepts = np.ascontiguousarray(
        np.broadcast_to(wrep[:, :, None], (128, P, NG))
    ).astype(np.float16)
    offs = np.tile(b3p * w_table.sum(axis=1), 4).reshape(128, 1).astype(np.float32)

    in_maps = []
    for c in range(NCORES):
        xs = x[c * E:(c + 1) * E]                                  # [E, 512]
        xt = np.ascontiguousarray(
            xs.reshape(E, P, F).transpose(2, 0, 1).reshape(F, E * P)
        ).astype(np.float16)
        es = eps[c * E:(c + 1) * E]                                # [E, 16]
        epst = np.ascontiguousarray(
            es.reshape(GALL, 4, LAT).transpose(1, 2, 0).reshape(64, GALL)
        )
        in_maps.append({
            "xt": xt, "w1": W1, "w2": W2, "w3": W3,
            "b1": b1, "b2": b2, "wrepts": wrepts,
            "offs": offs, "epst": epst,
        })
    return in_maps


def _host_post(results):
    mus = np.empty((B, LAT), np.float32)
    logvars = np.empty((B, LAT), np.float32)
    samples = np.empty((B, LAT), np.float32)
    for c, r in enumerate(results):
        for name, dst in (("mus_t", mus), ("logvars_t", logvars),
                          ("samples_t", samples)):
            t = r[name].reshape(4, LAT, GALL).transpose(2, 0, 1).reshape(E, LAT)
            dst[c * E:(c + 1) * E] = t
    return mus, logvars, samples


def kernel(**inputs):
    global LAST_RESULTS
    from concourse.bass_utils import run_bass_kernel_spmd

    nc = _build()
    in_maps = _host_prep(**inputs)
    trace = bool(int(os.environ.get("KERNEL_TRACE", "0")))
    res = run_bass_kernel_spmd(nc, in_maps, list(range(NCORES)), trace=trace)
    LAST_RESULTS = res
    return _host_post(res.results)


# revision 30
# speedup vs baseline: 1.0751x; 1.0071x over previous
"""Trainium2 Bass kernel for nn_Encoder (FSPool set encoder).

Computation per event b (8192 events, data-parallel over 8 cores):
  h = relu(x[b].reshape(128,4) @ W1 + b1)        # per-particle MLP
  h = relu(h @ W2 + b2)
  z = h @ W3 (+ b3)                              # [128 particles, 32 ch]
  z_sorted = sort_desc(z.T, axis=-1)             # per-channel sort over particles
  pooled[c] = sum_p z_sorted[c,p] * w[c,p]       # rank-weighted pool
  mus = pooled[::2]; logvars = pooled[1::2]
  samples = mus + eps * exp(0.5*logvars)

Device layout (per core, 1024 events, variable-size supertiles):
  - MLP on TensorE (hidden on partitions); relu+cast on ACT in 2-group
    (1024-col) PSUM tiles; z materialized channel-major via matmul3 with
    tile_position: partition = 32*(e%4)+c.
  - Per-row descending 128-sort: Batcher odd-even merge-sort (1471
    comparators vs bitonic's 1792) on the DVE in fp16, event "slots"
    packed along the inner free axis for 2x DVE mode. Inner merge passes
    touch only [d, m-d) of each block; untouched rows ping-pong via 4x
    tensor_copy.
  - Rank-weighted pooling (z_sorted*w then 7-level binary-tree sum over
    ranks) runs on GPSIMD, which has no other mid-kernel work; the last
    supertile's pooling runs on the DVE so the kernel doesn't drain
    waiting on the slower engine.
  - b3 never enters the sort: its pooled contribution is a per-partition
    offset added to the stage at the end.
"""

import os
import numpy as np

NCORES = 8
B = 8192
P = 128          # particles per event (set size)
F = 4            # input features per particle
H = 128          # hidden width
C = 32           # 2*LATENT pooled channels
LAT = 16
NPIECES = 20

E = B // NCORES          # events per core
ST_E = 128               # max events per supertile
NG = ST_E // 4           # max groups (slots) per supertile: 32
GALL = E // 4            # total groups per core (stage columns): 256

_BUILT = None
LAST_RESULTS = None      # test harness can inspect exec_time_ns / profile


def _chunks(e_total):
    """Event counts per super-tile: small head/tail tiles shorten the
    pipeline ramp-in and drain."""
    if e_total >= 8 * ST_E:
        q3 = 3 * ST_E // 4
        mid = (e_total - ST_E // 4 - ST_E // 2 - 2 * q3) // ST_E
        rem = e_total - ST_E // 4 - ST_E // 2 - 2 * q3 - mid * ST_E
        return [ST_E // 4, ST_E // 2, q3] + [ST_E] * mid + [rem, q3]
    out = []
    left = e_total
    while left > 0:
        c = min(ST_E, left)
        out.append(c)
        left -= c
    return out


def _fspool_interp_matrix():
    """M [21, 128] with w_table = pool_weight @ M (matches reference math)."""
    pos = (np.arange(P, dtype=np.float32) / np.float32(P - 1)) * np.float32(NPIECES)
    idx = np.clip(pos.astype(np.int32), 0, NPIECES)
    frac = pos - idx.astype(np.float32)
    M = np.zeros((NPIECES + 1, P), dtype=np.float32)
    for p in range(P):
        i = int(idx[p])
        M[i, p] += np.float32(1.0) - frac[p]
        M[min(i + 1, NPIECES), p] += frac[p]
    return M


def _batcher_passes(n=P, skip_final_d1=True):
    """(kind, m, d) pass list for Batcher odd-even mergesort of n.

    With skip_final_d1, the last cleanup pass of the final merge is
    dropped: the result is sorted except that pairs (2k+1, 2k+2) may be
    swapped. The pooling weight table is pair-averaged over exactly
    those pairs (host side), which makes the pooled sum invariant to
    the missing pass up to the (tiny) within-pair weight variation."""
    out = []
    m = 2
    while m <= n:
        out.append(("first", m, m // 2))
        d = m // 4
        while d >= 1:
            if not (skip_final_d1 and m == n and d == 1):
                out.append(("inner", m, d))
            d //= 2
        m *= 2
    return out


def _emit_batcher(v, OP, zA, zB, n=P):
    """Descending Batcher sort of the position axis of zA [128, P, s].
    Ping-pongs zA/zB every pass (28 passes, even -> result in zA)."""
    cur, other = zA, zB
    for kind, m, d in _batcher_passes(n):
        va = cur[:].rearrange("p (nb m) s -> p nb m s", m=m)
        vb = other[:].rearrange("p (nb m) s -> p nb m s", m=m)
        if kind == "first":
            v.tensor_tensor(out=vb[:, :, 0:d, :], in0=va[:, :, 0:d, :],
                            in1=va[:, :, d:m, :], op=OP.max)
            v.tensor_tensor(out=vb[:, :, d:m, :], in0=va[:, :, 0:d, :],
                            in1=va[:, :, d:m, :], op=OP.min)
        else:
            nruns = m // (2 * d) - 1
            # runs start at d + 2dt: view run axis explicitly
            ra = va[:].rearrange("p nb (r q) s -> p nb r q s", q=2 * d)
            rb = vb[:].rearrange("p nb (r q) s -> p nb r q s", q=2 * d)
            # compare (i, i+d) for i = d + 2dt
            v.tensor_tensor(
                out=rb[:, :, 0:nruns, d:2 * d, :],
                in0=ra[:, :, 0:nruns, d:2 * d, :],
                in1=rb0_in1(ra, nruns, d),
                op=OP.max,
            )
            v.tensor_tensor(
                out=rb[:, :, 1:nruns + 1, 0:d, :],
                in0=ra[:, :, 0:nruns, d:2 * d, :],
                in1=ra[:, :, 1:nruns + 1, 0:d, :],
                op=OP.min,
            )
            # Untouched boundary rows [0, d) and [m-d, m). Only the FIRST
            # inner pass of a merge level must copy them: its copy makes
            # the two ping-pong buffers agree on those rows, later passes
            # have strictly nested (and never again touched) boundaries, so
            # the stale buffer already holds the correct values.
            # For m >= 16 the copied rows are next read only at the next
            # merge's first pass, several DVE passes later — enough slack
            # for the copy to run on the mostly-idle GPSIMD (as x+0, the
            # proven tensor_scalar ucode) instead of costing DVE cycles.
            if d == m // 4:
                if m >= 64:
                    g = v.bass.gpsimd
                    g.tensor_scalar_add(vb[:, :, 0:d, :], va[:, :, 0:d, :], 0.0)
                    g.tensor_scalar_add(vb[:, :, m - d:m, :],
                                        va[:, :, m - d:m, :], 0.0)
                else:
                    v.tensor_copy(vb[:, :, 0:d, :], va[:, :, 0:d, :])
                    v.tensor_copy(vb[:, :, m - d:m, :], va[:, :, m - d:m, :])
        cur, other = other, cur
    return cur


def rb0_in1(ra, nruns, d):
    return ra[:, :, 1:nruns + 1, 0:d, :]


def _emit_tree_reduce(eng, OP, prod, scr, stage_slice, ns):
    """Sum over the P (rank) axis of prod [128, P, ns] via 7 binary-tree
    TT-add levels (fp16), final level fp32 into stage."""
    cur, other = prod, scr
    w = 64
    while w >= 2:
        eng.tensor_tensor(
            out=other[:, 0:w, :], in0=cur[:, 0:w, :],
            in1=cur[:, w:2 * w, :], op=OP.add,
        )
        cur, other = other, cur
        w //= 2
    eng.tensor_tensor(
        out=stage_slice, in0=cur[:, 0:1, :], in1=cur[:, 1:2, :], op=OP.add,
    )


def _build():
    global _BUILT
    if _BUILT is not None:
        return _BUILT
    from contextlib import ExitStack
    import concourse.bass as bass
    import concourse.bacc as bacc
    import concourse.tile as tile
    import concourse.mybir as mybir

    f32 = mybir.dt.float32
    f16 = mybir.dt.float16
    AF = mybir.ActivationFunctionType
    OP = mybir.AluOpType

    nc = bacc.Bacc("TRN2", target_bir_lowering=False, debug=False)

    xt_d = nc.dram_tensor("xt", [F, E * P], f16, kind="ExternalInput")
    w1_d = nc.dram_tensor("w1", [F, H], f16, kind="ExternalInput")
    w2_d = nc.dram_tensor("w2", [H, H], f16, kind="ExternalInput")
    w3_d = nc.dram_tensor("w3", [H, C], f16, kind="ExternalInput")
    b1_d = nc.dram_tensor("b1", [H, 1], f32, kind="ExternalInput")
    b2_d = nc.dram_tensor("b2", [H, 1], f32, kind="ExternalInput")
    # weight table, rank-major with slot-broadcast: [128, P, NG]
    wrepts_d = nc.dram_tensor("wrepts", [128, P, NG], f16, kind="ExternalInput")
    offs_d = nc.dram_tensor("offs", [128, 1], f32, kind="ExternalInput")
    epst_d = nc.dram_tensor("epst", [64, GALL], f32, kind="ExternalInput")

    mus_d = nc.dram_tensor("mus_t", [64, GALL], f32, kind="ExternalOutput")
    lv_d = nc.dram_tensor("logvars_t", [64, GALL], f32, kind="ExternalOutput")
    smp_d = nc.dram_tensor("samples_t", [64, GALL], f32, kind="ExternalOutput")

    with tile.TileContext(nc) as tc:
        with ExitStack() as ctx:
            consts = ctx.enter_context(tc.tile_pool(name="consts", bufs=1))
            xpool = ctx.enter_context(tc.tile_pool(name="x", bufs=2))
            hpool = ctx.enter_context(tc.tile_pool(name="h", bufs=3))
            zap = ctx.enter_context(tc.tile_pool(name="za", bufs=3))
            zbp = ctx.enter_context(tc.tile_pool(name="zb", bufs=3))
            prp = ctx.enter_context(tc.tile_pool(name="prod", bufs=2))
            spool = ctx.enter_context(tc.tile_pool(name="stage", bufs=1))
            epool = ctx.enter_context(tc.tile_pool(name="epi", bufs=1))
            ps1 = ctx.enter_context(tc.tile_pool(name="ps1", bufs=2, space="PSUM"))
            ps2 = ctx.enter_context(tc.tile_pool(name="ps2", bufs=1, space="PSUM"))
            ps3 = ctx.enter_context(tc.tile_pool(name="ps3", bufs=2, space="PSUM"))

            w1_s = consts.tile([F, H], f16)
            nc.sync.dma_start(out=w1_s[:], in_=w1_d[:])
            w2_s = consts.tile([H, H], f16)
            nc.sync.dma_start(out=w2_s[:], in_=w2_d[:])
            w3_s = consts.tile([H, C], f16)
            nc.sync.dma_start(out=w3_s[:], in_=w3_d[:])
            b1_s = consts.tile([H, 1], f32)
            nc.sync.dma_start(out=b1_s[:], in_=b1_d[:])
            b2_s = consts.tile([H, 1], f32)
            nc.sync.dma_start(out=b2_s[:], in_=b2_d[:])
            # wrepts (1 MB) is not needed until the first pooling ~60us in;
            # its dma_start is issued after chunk 0's input DMA so the first
            # MLP tile isn't queued behind it.
            wrepts_s = consts.tile([128, P, NG], f16)
            offs_s = consts.tile([128, 1], f32)

            stage = spool.tile([128, GALL], f32)
            # assembled per chunk: channels host-permuted so that within
            # each 32-partition stage block, 0:16 are mus, 16:32 logvars
            mus_al = epool.tile([64, GALL], f32)
            lv_al = epool.tile([64, GALL], f32)

            chunks = _chunks(E)
            ev0 = 0
            for st, st_e in enumerate(chunks):
                ng = st_e // 4
                ngq = ng // 4
                col0 = ev0 // 4
                last = st == len(chunks) - 1
                ramp = st == 0
                xt_s = xpool.tile([F, st_e * P], f16, tag="xt")
                nc.sync.dma_start(
                    out=xt_s[:], in_=xt_d[:, ev0 * P:(ev0 + st_e) * P]
                )
                if st == 0:
                    nc.sync.dma_start(out=wrepts_s[:], in_=wrepts_d[:])
                    nc.sync.dma_start(out=offs_s[:], in_=offs_d[:])
                zA = zap.tile([128, P, ng], f16, tag="zA")
                zB = zbp.tile([128, P, ng], f16, tag="zB")

                for gq in range(ngq):
                    pz = ps3.tile([128, 4 * P], f32, tag="pz")
                    for half in range(2):
                        g2 = gq * 4 + half * 2        # first of 2 groups
                        ph1 = ps1.tile([128, 1024], f32, tag="ph1")
                        for gg in range(2):
                            nc.tensor.matmul(
                                ph1[:, gg * 512:(gg + 1) * 512], w1_s[:],
                                xt_s[:, (g2 + gg) * 512:(g2 + gg + 1) * 512],
                                start=True, stop=True,
                            )
                        h1 = hpool.tile([128, 1024], f16, tag="h1")
                        if ramp and half == 0:
                            nc.vector.tensor_scalar(
                                out=h1[:], in0=ph1[:], scalar1=b1_s[:],
                                scalar2=0.0, op0=OP.add, op1=OP.max)
                        else:
                            nc.scalar.activation(h1[:], ph1[:], AF.Relu,
                                                 bias=b1_s[:])
                        ph2 = ps2.tile([128, 1024], f32, tag="ph2")
                        for gg in range(2):
                            nc.tensor.matmul(
                                ph2[:, gg * 512:(gg + 1) * 512], w2_s[:],
                                h1[:, gg * 512:(gg + 1) * 512],
                                start=True, stop=True)
                        h2 = hpool.tile([128, 1024], f16, tag="h2")
                        if ramp and half == 1:
                            nc.vector.tensor_scalar(
                                out=h2[:], in0=ph2[:], scalar1=b2_s[:],
                                scalar2=0.0, op0=OP.add, op1=OP.max)
                        else:
                            nc.scalar.activation(h2[:], ph2[:], AF.Relu,
                                                 bias=b2_s[:])
                        for gg in range(2):
                            for e4 in range(4):
                                nc.tensor.matmul(
                                    pz[32 * e4:32 * (e4 + 1),
                                       (half * 2 + gg) * P:(half * 2 + gg + 1) * P],
                                    w3_s[:],
                                    h2[:, (gg * 4 + e4) * P:(gg * 4 + e4 + 1) * P],
                                    start=True, stop=True,
                                    tile_position=(0, 32 * e4),
                                )
                    nc.scalar.activation(
                        zA[:, :, 4 * gq:4 * (gq + 1)].rearrange("a p s -> a s p"),
                        pz[:].rearrange("a (s p) -> a s p", p=P),
                        AF.Copy,
                    )

                # ---- sort (DVE) ----
                cur = _emit_batcher(nc.vector, OP, zA, zB)

                # ---- rank-weighted pooling ----
                # GPSIMD normally; DVE for the last chunk (avoids draining
                # behind the slower engine at the end).
                eng = nc.vector if last else nc.gpsimd
                prod = prp.tile([128, P, ng], f16, tag="prod")
                eng.tensor_tensor(
                    out=prod[:], in0=cur[:], in1=wrepts_s[:, :, 0:ng],
                    op=OP.mult,
                )
                scr = prp.tile([128, 64, ng], f16, tag="scr")
                _emit_tree_reduce(
                    eng, OP, prod, scr,
                    stage[:, col0:col0 + ng].rearrange("a (o s) -> a o s", o=1),
                    ng,
                )
                # b3's pooled offset + mus/lv assembly, pipelined per chunk
                # so the epilogue isn't one serial chain after the last sort
                eng.tensor_scalar_add(stage[:, col0:col0 + ng],
                                      stage[:, col0:col0 + ng], offs_s[:])
                for q in range(4):
                    nc.sync.dma_start(
                        out=mus_al[16 * q:16 * (q + 1), col0:col0 + ng],
                        in_=stage[32 * q:32 * q + 16, col0:col0 + ng],
                    )
                    nc.sync.dma_start(
                        out=lv_al[16 * q:16 * (q + 1), col0:col0 + ng],
                        in_=stage[32 * q + 16:32 * q + 32, col0:col0 + ng],
                    )
                ev0 += st_e

            # ---- epilogue (mus_al/lv_al assembled per chunk above) ----
            ex = epool.tile([64, GALL], f32)
            nc.scalar.activation(ex[:], lv_al[:], AF.Exp, scale=0.5)
            eps_s = epool.tile([64, GALL], f32)
            nc.sync.dma_start(out=eps_s[:], in_=epst_d[:])
            smp = epool.tile([64, GALL], f32)
            nc.vector.tensor_tensor(out=smp[:], in0=eps_s[:], in1=ex[:], op=OP.mult)
            nc.vector.tensor_tensor(out=smp[:], in0=smp[:], in1=mus_al[:], op=OP.add)
            nc.sync.dma_start(out=mus_d[:], in_=mus_al[:])
            nc.sync.dma_start(out=lv_d[:], in_=lv_al[:])
            nc.sync.dma_start(out=smp_d[:], in_=smp[:])

    nc.compile()
    _BUILT = nc
    return nc


def _host_prep(x, W1, b1, W2, b2, W3, b3, pool_weight, eps):
    x = np.asarray(x, np.float32)
    eps = np.asarray(eps, np.float32)
    W1 = np.asarray(W1, np.float32).astype(np.float16)
    W2 = np.asarray(W2, np.float32).astype(np.float16)
    W3 = np.asarray(W3, np.float32)
    b1 = np.asarray(b1, np.float32).reshape(H, 1)
    b2 = np.asarray(b2, np.float32).reshape(H, 1)
    b3 = np.asarray(b3, np.float32)
    pw = np.asarray(pool_weight, np.float32)

    # channel permutation: device channel c' maps to logical channel perm[c']
    # (mus channels 0,2,..,30 first, then logvar channels 1,3,..,31)
    perm = np.concatenate([np.arange(0, C, 2), np.arange(1, C, 2)])
    W3 = np.ascontiguousarray(W3[:, perm]).astype(np.float16)
    b3p = b3[perm]
    w_table = (pw @ _fspool_interp_matrix()).astype(np.float32)[perm]  # [32, 128]
    # pair-average ranks (1,2),(3,4),...,(125,126): the device skips the
    # final d=1 merge pass, leaving exactly these pairs possibly swapped.
    wp = w_table.copy()
    pav = 0.5 * (w_table[:, 1:127:2] + w_table[:, 2:127:2])
    wp[:, 1:127:2] = pav
    wp[:, 2:127:2] = pav
    w_table_dev = wp
    wrep = np.tile(w_table_dev, (4, 1))                                # [128, 128]
    wrepts = np.ascontiguousarray(
        np.broadcast_to(wrep[:, :, None], (128, P, NG))
    ).astype(np.float16)
    offs = np.tile(b3p * w_table.sum(axis=1), 4).reshape(128, 1).astype(np.float32)

    in_maps = []
    for c in range(NCORES):
        xs = x[c * E:(c + 1) * E]                                  # [E, 512]
        xt = np.ascontiguousarray(
            xs.reshape(E, P, F).transpose(2, 0, 1).reshape(F, E * P)
        ).astype(np.float16)
        es = eps[c * E:(c + 1) * E]                                # [E, 16]
        epst = np.ascontiguousarray(
            es.reshape(GALL, 4, LAT).transpose(1, 2, 0).reshape(64, GALL)
        )
        in_maps.append({
            "xt": xt, "w1": W1, "w2": W2, "w3": W3,
            "b1": b1, "b2": b2, "wrepts": wrepts,
            "offs": offs, "epst": epst,
        })
    return in_maps


def _host_post(results):
    mus = np.empty((B, LAT), np.float32)
    logvars = np.empty((B, LAT), np.float32)
    samples = np.empty((B, LAT), np.float32)
    for c, r in enumerate(results):
        for name, dst in (("mus_t", mus), ("logvars_t", logvars),
                          ("samples_t", samples)):
            t = r[name].reshape(4, LAT, GALL).transpose(2, 0, 1).reshape(E, LAT)
            dst[c * E:(c + 1) * E] = t
    return mus, logvars, samples


def kernel(**inputs):
    global LAST_RESULTS
    from concourse.bass_utils import run_bass_kernel_spmd

    nc = _build()
    in_maps = _host_prep(**inputs)
    trace = bool(int(os.environ.get("KERNEL_TRACE", "0")))
    res = run_bass_kernel_spmd(nc, in_maps, list(range(NCORES)), trace=trace)
    LAST_RESULTS = res
    return _host_post(res.results)


# revision 37
# speedup vs baseline: 1.0764x; 1.0012x over previous
"""Trainium2 Bass kernel for nn_Encoder (FSPool set encoder).

Computation per event b (8192 events, data-parallel over 8 cores):
  h = relu(x[b].reshape(128,4) @ W1 + b1)        # per-particle MLP
  h = relu(h @ W2 + b2)
  z = h @ W3 (+ b3)                              # [128 particles, 32 ch]
  z_sorted = sort_desc(z.T, axis=-1)             # per-channel sort over particles
  pooled[c] = sum_p z_sorted[c,p] * w[c,p]       # rank-weighted pool
  mus = pooled[::2]; logvars = pooled[1::2]
  samples = mus + eps * exp(0.5*logvars)

Device layout (per core, 1024 events, variable-size supertiles):
  - MLP on TensorE (hidden on partitions); relu+cast on ACT in 2-group
    (1024-col) PSUM tiles; z materialized channel-major via matmul3 with
    tile_position: partition = 32*(e%4)+c.
  - Per-row descending 128-sort: Batcher odd-even merge-sort (1471
    comparators vs bitonic's 1792) on the DVE in fp16, event "slots"
    packed along the inner free axis for 2x DVE mode. Inner merge passes
    touch only [d, m-d) of each block; untouched rows ping-pong via 4x
    tensor_copy.
  - Rank-weighted pooling (z_sorted*w then 7-level binary-tree sum over
    ranks) runs on GPSIMD, which has no other mid-kernel work; the last
    supertile's pooling runs on the DVE so the kernel doesn't drain
    waiting on the slower engine.
  - b3 never enters the sort: its pooled contribution is a per-partition
    offset added to the stage at the end.
"""

import os
import numpy as np

NCORES = 8
B = 8192
P = 128          # particles per event (set size)
F = 4            # input features per particle
H = 128          # hidden width
C = 32           # 2*LATENT pooled channels
LAT = 16
NPIECES = 20

E = B // NCORES          # events per core
ST_E = 128               # max events per supertile
NG = ST_E // 4           # max groups (slots) per supertile: 32
GALL = E // 4            # total groups per core (stage columns): 256

OVERLAP = 8              # sort passes interleaved across adjacent chunks

_BUILT = None
LAST_RESULTS = None      # test harness can inspect exec_time_ns / profile


def _chunks(e_total):
    """Event counts per super-tile: small head/tail tiles shorten the
    pipeline ramp-in and drain."""
    if e_total >= 8 * ST_E:
        q3 = 3 * ST_E // 4
        mid = (e_total - ST_E // 4 - ST_E // 2 - 2 * q3) // ST_E
        rem = e_total - ST_E // 4 - ST_E // 2 - 2 * q3 - mid * ST_E
        return [ST_E // 4, ST_E // 2, q3] + [ST_E] * mid + [rem, q3]
    out = []
    left = e_total
    while left > 0:
        c = min(ST_E, left)
        out.append(c)
        left -= c
    return out


def _fspool_interp_matrix():
    """M [21, 128] with w_table = pool_weight @ M (matches reference math)."""
    pos = (np.arange(P, dtype=np.float32) / np.float32(P - 1)) * np.float32(NPIECES)
    idx = np.clip(pos.astype(np.int32), 0, NPIECES)
    frac = pos - idx.astype(np.float32)
    M = np.zeros((NPIECES + 1, P), dtype=np.float32)
    for p in range(P):
        i = int(idx[p])
        M[i, p] += np.float32(1.0) - frac[p]
        M[min(i + 1, NPIECES), p] += frac[p]
    return M


def _batcher_passes(n=P, skip_final_d1=True):
    """(kind, m, d) pass list for Batcher odd-even mergesort of n.

    With skip_final_d1, the last cleanup pass of the final merge is
    dropped: the result is sorted except that pairs (2k+1, 2k+2) may be
    swapped. The pooling weight table is pair-averaged over exactly
    those pairs (host side), which makes the pooled sum invariant to
    the missing pass up to the (tiny) within-pair weight variation."""
    out = []
    m = 2
    while m <= n:
        out.append(("first", m, m // 2))
        d = m // 4
        while d >= 1:
            if not (skip_final_d1 and m == n and d == 1):
                out.append(("inner", m, d))
            d //= 2
        m *= 2
    return out


def _batcher_closures(v, OP, zA, zB, n=P):
    """Emission closures (one per pass) for the descending Batcher sort of
    zA's position axis, plus the tile holding the final result. Deferred
    emission lets the build loop software-pipeline two chunks' sorts on
    the DVE (independent passes hide per-pass data-ready stalls)."""
    closures = []
    cur, other = zA, zB
    for kind, m, d in _batcher_passes(n):
        closures.append(_make_pass(v, OP, cur, other, kind, m, d))
        cur, other = other, cur
    return closures, cur


def _make_pass(v, OP, cur, other, kind, m, d):
    def emit():
        va = cur[:].rearrange("p (nb m) s -> p nb m s", m=m)
        vb = other[:].rearrange("p (nb m) s -> p nb m s", m=m)
        if kind == "first":
            v.tensor_tensor(out=vb[:, :, 0:d, :], in0=va[:, :, 0:d, :],
                            in1=va[:, :, d:m, :], op=OP.max)
            v.tensor_tensor(out=vb[:, :, d:m, :], in0=va[:, :, 0:d, :],
                            in1=va[:, :, d:m, :], op=OP.min)
        else:
            nruns = m // (2 * d) - 1
            # runs start at d + 2dt: view run axis explicitly
            ra = va[:].rearrange("p nb (r q) s -> p nb r q s", q=2 * d)
            rb = vb[:].rearrange("p nb (r q) s -> p nb r q s", q=2 * d)
            # compare (i, i+d) for i = d + 2dt
            v.tensor_tensor(
                out=rb[:, :, 0:nruns, d:2 * d, :],
                in0=ra[:, :, 0:nruns, d:2 * d, :],
                in1=ra[:, :, 1:nruns + 1, 0:d, :],
                op=OP.max,
            )
            v.tensor_tensor(
                out=rb[:, :, 1:nruns + 1, 0:d, :],
                in0=ra[:, :, 0:nruns, d:2 * d, :],
                in1=ra[:, :, 1:nruns + 1, 0:d, :],
                op=OP.min,
            )
            # Untouched boundary rows [0, d) and [m-d, m). Only the FIRST
            # inner pass of a merge level must copy them: its copy makes
            # the two ping-pong buffers agree on those rows, later passes
            # have strictly nested (and never again touched) boundaries, so
            # the stale buffer already holds the correct values.
            # For m >= 64 the copied rows are next read only several DVE
            # passes later — enough slack for the copy to run on the
            # mostly-idle GPSIMD (as x+0, the proven tensor_scalar ucode)
            # instead of costing DVE cycles.
            if d == m // 4:
                if m >= 64:
                    g = v.bass.gpsimd
                    g.tensor_scalar_add(vb[:, :, 0:d, :], va[:, :, 0:d, :], 0.0)
                    g.tensor_scalar_add(vb[:, :, m - d:m, :],
                                        va[:, :, m - d:m, :], 0.0)
                else:
                    v.tensor_copy(vb[:, :, 0:d, :], va[:, :, 0:d, :])
                    v.tensor_copy(vb[:, :, m - d:m, :], va[:, :, m - d:m, :])
    return emit


def _emit_tree_reduce(eng, OP, prod, scr, stage_slice, ns):
    """Sum over the P (rank) axis of prod [128, P, ns] via 7 binary-tree
    TT-add levels (fp16), final level fp32 into stage."""
    cur, other = prod, scr
    w = 64
    while w >= 2:
        eng.tensor_tensor(
            out=other[:, 0:w, :], in0=cur[:, 0:w, :],
            in1=cur[:, w:2 * w, :], op=OP.add,
        )
        cur, other = other, cur
        w //= 2
    eng.tensor_tensor(
        out=stage_slice, in0=cur[:, 0:1, :], in1=cur[:, 1:2, :], op=OP.add,
    )


def _build():
    global _BUILT
    if _BUILT is not None:
        return _BUILT
    from contextlib import ExitStack
    import concourse.bass as bass
    import concourse.bacc as bacc
    import concourse.tile as tile
    import concourse.mybir as mybir

    f32 = mybir.dt.float32
    f16 = mybir.dt.float16
    AF = mybir.ActivationFunctionType
    OP = mybir.AluOpType

    nc = bacc.Bacc("TRN2", target_bir_lowering=False, debug=False)

    xt_d = nc.dram_tensor("xt", [F, E * P], f16, kind="ExternalInput")
    w1_d = nc.dram_tensor("w1", [F, H], f16, kind="ExternalInput")
    w2_d = nc.dram_tensor("w2", [H, H], f16, kind="ExternalInput")
    w3_d = nc.dram_tensor("w3", [H, C], f16, kind="ExternalInput")
    b1_d = nc.dram_tensor("b1", [H, 1], f32, kind="ExternalInput")
    b2_d = nc.dram_tensor("b2", [H, 1], f32, kind="ExternalInput")
    # weight table, rank-major with slot-broadcast: [128, P, NG]
    wrepts_d = nc.dram_tensor("wrepts", [128, P, NG], f16, kind="ExternalInput")
    offs_d = nc.dram_tensor("offs", [128, 1], f32, kind="ExternalInput")
    epst_d = nc.dram_tensor("epst", [64, GALL], f32, kind="ExternalInput")

    mus_d = nc.dram_tensor("mus_t", [64, GALL], f32, kind="ExternalOutput")
    lv_d = nc.dram_tensor("logvars_t", [64, GALL], f32, kind="ExternalOutput")
    smp_d = nc.dram_tensor("samples_t", [64, GALL], f32, kind="ExternalOutput")

    with tile.TileContext(nc) as tc:
        with ExitStack() as ctx:
            consts = ctx.enter_context(tc.tile_pool(name="consts", bufs=1))
            xpool = ctx.enter_context(tc.tile_pool(name="x", bufs=2))
            hpool = ctx.enter_context(tc.tile_pool(name="h", bufs=3))
            zap = ctx.enter_context(tc.tile_pool(name="za", bufs=3))
            zbp = ctx.enter_context(tc.tile_pool(name="zb", bufs=3))
            prp = ctx.enter_context(tc.tile_pool(name="prod", bufs=2))
            spool = ctx.enter_context(tc.tile_pool(name="stage", bufs=1))
            epool = ctx.enter_context(tc.tile_pool(name="epi", bufs=1))
            ps1 = ctx.enter_context(tc.tile_pool(name="ps1", bufs=2, space="PSUM"))
            ps2 = ctx.enter_context(tc.tile_pool(name="ps2", bufs=1, space="PSUM"))
            ps3 = ctx.enter_context(tc.tile_pool(name="ps3", bufs=2, space="PSUM"))

            w1_s = consts.tile([F, H], f16)
            nc.sync.dma_start(out=w1_s[:], in_=w1_d[:])
            w2_s = consts.tile([H, H], f16)
            nc.sync.dma_start(out=w2_s[:], in_=w2_d[:])
            w3_s = consts.tile([H, C], f16)
            nc.sync.dma_start(out=w3_s[:], in_=w3_d[:])
            b1_s = consts.tile([H, 1], f32)
            nc.sync.dma_start(out=b1_s[:], in_=b1_d[:])
            b2_s = consts.tile([H, 1], f32)
            nc.sync.dma_start(out=b2_s[:], in_=b2_d[:])
            # wrepts (1 MB) is not needed until the first pooling ~60us in;
            # its dma_start is issued after chunk 0's input DMA so the first
            # MLP tile isn't queued behind it.
            wrepts_s = consts.tile([128, P, NG], f16)
            offs_s = consts.tile([128, 1], f32)

            stage = spool.tile([128, GALL], f32)
            # assembled per chunk: channels host-permuted so that within
            # each 32-partition stage block, 0:16 are mus, 16:32 logvars
            mus_al = epool.tile([64, GALL], f32)
            lv_al = epool.tile([64, GALL], f32)

            chunks = _chunks(E)
            ev0 = 0
            pending = None
            for st, st_e in enumerate(chunks):
                ng = st_e // 4
                ngq = ng // 4
                col0 = ev0 // 4
                last = st == len(chunks) - 1
                ramp = st == 0
                xt_s = xpool.tile([F, st_e * P], f16, tag="xt")
                nc.sync.dma_start(
                    out=xt_s[:], in_=xt_d[:, ev0 * P:(ev0 + st_e) * P]
                )
                if st == 0:
                    nc.sync.dma_start(out=wrepts_s[:], in_=wrepts_d[:])
                    nc.sync.dma_start(out=offs_s[:], in_=offs_d[:])
                zA = zap.tile([128, P, ng], f16, tag="zA")
                zB = zbp.tile([128, P, ng], f16, tag="zB")

                for gq in range(ngq):
                    pz = ps3.tile([128, 4 * P], f32, tag="pz")
                    for half in range(2):
                        g2 = gq * 4 + half * 2        # first of 2 groups
                        ph1 = ps1.tile([128, 1024], f32, tag="ph1")
                        for gg in range(2):
                            nc.tensor.matmul(
                                ph1[:, gg * 512:(gg + 1) * 512], w1_s[:],
                                xt_s[:, (g2 + gg) * 512:(g2 + gg + 1) * 512],
                                start=True, stop=True,
                            )
                        h1 = hpool.tile([128, 1024], f16, tag="h1")
                        if ramp and half == 0:
                            nc.vector.tensor_scalar(
                                out=h1[:], in0=ph1[:], scalar1=b1_s[:],
                                scalar2=0.0, op0=OP.add, op1=OP.max)
                        else:
                            nc.scalar.activation(h1[:], ph1[:], AF.Relu,
                                                 bias=b1_s[:])
                        ph2 = ps2.tile([128, 1024], f32, tag="ph2")
                        for gg in range(2):
                            nc.tensor.matmul(
                                ph2[:, gg * 512:(gg + 1) * 512], w2_s[:],
                                h1[:, gg * 512:(gg + 1) * 512],
                                start=True, stop=True)
                        h2 = hpool.tile([128, 1024], f16, tag="h2")
                        if ramp and half == 1:
                            nc.vector.tensor_scalar(
                                out=h2[:], in0=ph2[:], scalar1=b2_s[:],
                                scalar2=0.0, op0=OP.add, op1=OP.max)
                        else:
                            nc.scalar.activation(h2[:], ph2[:], AF.Relu,
                                                 bias=b2_s[:])
                        for gg in range(2):
                            for e4 in range(4):
                                nc.tensor.matmul(
                                    pz[32 * e4:32 * (e4 + 1),
                                       (half * 2 + gg) * P:(half * 2 + gg + 1) * P],
                                    w3_s[:],
                                    h2[:, (gg * 4 + e4) * P:(gg * 4 + e4 + 1) * P],
                                    start=True, stop=True,
                                    tile_position=(0, 32 * e4),
                                )
                    nc.scalar.activation(
                        zA[:, :, 4 * gq:4 * (gq + 1)].rearrange("a p s -> a s p"),
                        pz[:].rearrange("a (s p) -> a s p", p=P),
                        AF.Copy,
                    )

                # ---- sort (DVE), software-pipelined across chunks ----
                # The tail passes of the previous chunk's sort interleave
                # with this chunk's head passes: independent ops fill the
                # per-pass data-ready stalls on the in-order DVE.
                plist, cur = _batcher_closures(nc.vector, OP, zA, zB)

                def make_pool(cur=cur, ng=ng, col0=col0, last=last):
                    def emit():
                        # GPSIMD normally; DVE for the last chunk (avoids
                        # draining behind the slower engine at the end)
                        eng = nc.vector if last else nc.gpsimd
                        prod = prp.tile([128, P, ng], f16, tag="prod")
                        eng.tensor_tensor(
                            out=prod[:], in0=cur[:], in1=wrepts_s[:, :, 0:ng],
                            op=OP.mult,
                        )
                        scr = prp.tile([128, 64, ng], f16, tag="scr")
                        _emit_tree_reduce(
                            eng, OP, prod, scr,
                            stage[:, col0:col0 + ng].rearrange(
                                "a (o s) -> a o s", o=1),
                            ng,
                        )
                        # b3's pooled offset + mus/lv assembly, per chunk so
                        # the epilogue isn't one serial chain at the end
                        eng.tensor_scalar_add(stage[:, col0:col0 + ng],
                                              stage[:, col0:col0 + ng],
                                              offs_s[:])
                        for q in range(4):
                            nc.sync.dma_start(
                                out=mus_al[16 * q:16 * (q + 1), col0:col0 + ng],
                                in_=stage[32 * q:32 * q + 16, col0:col0 + ng],
                            )
                            nc.sync.dma_start(
                                out=lv_al[16 * q:16 * (q + 1), col0:col0 + ng],
                                in_=stage[32 * q + 16:32 * q + 32,
                                          col0:col0 + ng],
                            )
                    return emit

                if pending is None:
                    for fn in plist[:-OVERLAP]:
                        fn()
                else:
                    prev_tail, prev_pool = pending
                    for a, b in zip(prev_tail, plist[:OVERLAP]):
                        a()
                        b()
                    prev_pool()
                    for fn in plist[OVERLAP:-OVERLAP]:
                        fn()
                pending = (plist[-OVERLAP:], make_pool())
                ev0 += st_e

            prev_tail, prev_pool = pending
            for fn in prev_tail:
                fn()
            prev_pool()

            # ---- epilogue (mus_al/lv_al assembled per chunk above) ----
            ex = epool.tile([64, GALL], f32)
            nc.scalar.activation(ex[:], lv_al[:], AF.Exp, scale=0.5)
            eps_s = epool.tile([64, GALL], f32)
            nc.sync.dma_start(out=eps_s[:], in_=epst_d[:])
            smp = epool.tile([64, GALL], f32)
            nc.vector.tensor_tensor(out=smp[:], in0=eps_s[:], in1=ex[:], op=OP.mult)
            nc.vector.tensor_tensor(out=smp[:], in0=smp[:], in1=mus_al[:], op=OP.add)
            nc.sync.dma_start(out=mus_d[:], in_=mus_al[:])
            nc.sync.dma_start(out=lv_d[:], in_=lv_al[:])
            nc.sync.dma_start(out=smp_d[:], in_=smp[:])

    nc.compile()
    _BUILT = nc
    return nc


def _host_prep(x, W1, b1, W2, b2, W3, b3, pool_weight, eps):
    x = np.asarray(x, np.float32)
    eps = np.asarray(eps, np.float32)
    W1 = np.asarray(W1, np.float32).astype(np.float16)
    W2 = np.asarray(W2, np.float32).astype(np.float16)
    W3 = np.asarray(W3, np.float32)
    b1 = np.asarray(b1, np.float32).reshape(H, 1)
    b2 = np.asarray(b2, np.float32).reshape(H, 1)
    b3 = np.asarray(b3, np.float32)
    pw = np.asarray(pool_weight, np.float32)

    # channel permutation: device channel c' maps to logical channel perm[c']
    # (mus channels 0,2,..,30 first, then logvar channels 1,3,..,31)
    perm = np.concatenate([np.arange(0, C, 2), np.arange(1, C, 2)])
    W3 = np.ascontiguousarray(W3[:, perm]).astype(np.float16)
    b3p = b3[perm]
    w_table = (pw @ _fspool_interp_matrix()).astype(np.float32)[perm]  # [32, 128]
    # pair-average ranks (1,2),(3,4),...,(125,126): the device skips the
    # final d=1 merge pass, leaving exactly these pairs possibly swapped.
    wp = w_table.copy()
    pav = 0.5 * (w_table[:, 1:127:2] + w_table[:, 2:127:2])
    wp[:, 1:127:2] = pav
    wp[:, 2:127:2] = pav
    w_table_dev = wp
    wrep = np.tile(w_table_dev, (4, 1))                                # [128, 128]
    wrepts = np.ascontiguousarray(
        np.broadcast_to(wrep[:, :, None], (128, P, NG))
    ).astype(np.float16)
    offs = np.tile(b3p * w_table.sum(axis=1), 4).reshape(128, 1).astype(np.float32)

    in_maps = []
    for c in range(NCORES):
        xs = x[c * E:(c + 1) * E]                                  # [E, 512]
        xt = np.ascontiguousarray(
            xs.reshape(E, P, F).transpose(2, 0, 1).reshape(F, E * P)
        ).astype(np.float16)
        es = eps[c * E:(c + 1) * E]                                # [E, 16]
        epst = np.ascontiguousarray(
            es.reshape(GALL, 4, LAT).transpose(1, 2, 0).reshape(64, GALL)
        )
        in_maps.append({
            "xt": xt, "w1": W1, "w2": W2, "w3": W3,
            "b1": b1, "b2": b2, "wrepts": wrepts,
            "offs": offs, "epst": epst,
        })
    return in_maps


def _host_post(results):
    mus = np.empty((B, LAT), np.float32)
    logvars = np.empty((B, LAT), np.float32)
    samples = np.empty((B, LAT), np.float32)
    for c, r in enumerate(results):
        for name, dst in (("mus_t", mus), ("logvars_t", logvars),
                          ("samples_t", samples)):
            t = r[name].reshape(4, LAT, GALL).transpose(2, 0, 1).reshape(E, LAT)
            dst[c * E:(c + 1) * E] = t
    return mus, logvars, samples


def kernel(**inputs):
    global LAST_RESULTS
    from concourse.bass_utils import run_bass_kernel_spmd

    nc = _build()
    in_maps = _host_prep(**inputs)
    trace = bool(int(os.environ.get("KERNEL_TRACE", "0")))
    res = run_bass_kernel_spmd(nc, in_maps, list(range(NCORES)), trace=trace)
    LAST_RESULTS = res
    return _host_post(res.results)
